# revision 10
# baseline (speedup 1.0000x reference)
"""Trainium2 Bass kernel for GQA attention (dense_transformer).

Sharding: 8 cores = 2-way data parallel (batch) x 4-way tensor parallel (heads).
Core c handles batch b=c//4, head group g=c%4 (8 q heads, 2 kv heads).
Per core: QKV projections (bf16, f32 accum), RoPE, causal attention with
transposed-scores layout (scoresT[k,q] -> probsT used directly as the moving
operand of the PV matmul; no on-chip transposes), per-chunk AllGather of
attention outputs within each 4-core group, then a column-sharded Wo matmul.
Host assembles disjoint output shards (pure unshard, no host math).
"""
import sys

sys.path.insert(0, "/opt/trn_rl_repo")

import numpy as np
import ml_dtypes

import concourse.bacc as bacc
import concourse.mybir as mybir
import concourse.tile as tile
from concourse import bass_utils

BF16 = ml_dtypes.bfloat16

B, S, HID = 2, 2048, 4096
NH, NKV, D = 32, 8, 128
NCORES, GRP = 8, 4          # 2 groups of 4 cores
QH, KVH = NH // GRP, NKV // GRP   # 8 q heads, 2 kv heads per core
QD, KVD = QH * D, KVH * D         # 1024, 256
CH, NCH = 512, S // 512           # q-chunk size / count
KB = 128                          # k block
NIT = HID // 128                  # 32 contraction tiles
SM_SCALE = float(D) ** -0.5
REPLICA_GROUPS = [[0, 1, 2, 3], [4, 5, 6, 7]]

_CACHE: dict = {}


# ---------------------------------------------------------------- builder
def build_nc(plan, nmask, debug_dumps=False, stages="all"):
    """plan[c] = list of (t, mask_idx_or_None) k-blocks for q-chunk c."""
    fp32, bf16, f32r = mybir.dt.float32, mybir.dt.bfloat16, mybir.dt.float32r
    nc = bacc.Bacc("TRN2", target_bir_lowering=False, debug=False,
                   num_devices=NCORES)

    xT = nc.dram_tensor("xT", [HID, S], bf16, kind="ExternalInput")
    wq = nc.dram_tensor("wq", [HID, QD], bf16, kind="ExternalInput")
    wk = nc.dram_tensor("wk", [HID, KVD], bf16, kind="ExternalInput")
    wv = nc.dram_tensor("wv", [HID, KVD], bf16, kind="ExternalInput")
    wo = nc.dram_tensor("wo", [HID, QD], bf16, kind="ExternalInput")
    cosT = nc.dram_tensor("cosT", [D, S], bf16, kind="ExternalInput")
    sinS = nc.dram_tensor("sinS", [D, S], bf16, kind="ExternalInput")
    dmask = nc.dram_tensor("dmask", [max(nmask, 1), KB, CH], bf16,
                           kind="ExternalInput")
    outT = nc.dram_tensor("outT", [QD, S], fp32, kind="ExternalOutput")

    dbg = {}
    if debug_dumps:
        dbg["qt0"] = nc.dram_tensor("dbg_qt0", [128, QH * CH], bf16,
                                    kind="ExternalOutput")
        dbg["kt0"] = nc.dram_tensor("dbg_kt0", [D, S], bf16,
                                    kind="ExternalOutput")
        dbg["v"] = nc.dram_tensor("dbg_v", [128, (S // 128) * KVD], bf16,
                                  kind="ExternalOutput")
        dbg["wk"] = nc.dram_tensor("dbg_wk", [128, NIT * KVD], bf16,
                                   kind="ExternalOutput")
        dbg["wv"] = nc.dram_tensor("dbg_wv", [128, NIT * KVD], bf16,
                                   kind="ExternalOutput")
        dbg["dm"] = nc.dram_tensor("dbg_dm", [KB, 4 * CH], bf16,
                                   kind="ExternalOutput")
        dbg["bin0"] = nc.dram_tensor("dbg_bin0", [QD, CH], bf16,
                                     kind="ExternalOutput")
        dbg["bout0"] = nc.dram_tensor("dbg_bout0", [HID, CH], bf16,
                                      kind="ExternalOutput")
    bnc_in = [nc.dram_tensor(f"bnc_in{c}", [QD, CH], bf16) for c in range(NCH)]
    bnc_out = [nc.dram_tensor(f"bnc_out{c}", [HID, CH], bf16) for c in range(NCH)]

    with tile.TileContext(nc) as tc:
        from contextlib import ExitStack
        with ExitStack() as ctx:
            P = lambda **kw: ctx.enter_context(tc.tile_pool(**kw))
            const_p = P(name="const", bufs=1)
            wkv_p = P(name="wkv", bufs=1)
            res_p = P(name="res", bufs=1)         # kT, v, cos, sin, masks
            xt_p = P(name="xt", bufs=1)
            wq_p = P(name="wqp", bufs=2)
            qt_p = P(name="qt", bufs=2)
            rope_p = P(name="rope", bufs=2)
            probs_p = P(name="probs", bufs=4)
            attn_p = P(name="attn", bufs=3)
            gath_p = P(name="gath", bufs=1)
            wo_p = P(name="wop", bufs=2)
            out_p = P(name="outp", bufs=2)
            recip_p = P(name="recip", bufs=2)
            pA = P(name="pA", bufs=2, space="PSUM")
            psc = P(name="psc", bufs=2, space="PSUM")
            po = P(name="po", bufs=2, space="PSUM")
            psums = P(name="psums", bufs=1, space="PSUM")
            pbc = P(name="pbc", bufs=1, space="PSUM")

            # constants
            ones_f = const_p.tile([1, 128], fp32, tag="ones_f")
            nc.gpsimd.memset(ones_f[:], 1.0)
            ones_r = const_p.tile([1, 128], f32r, tag="ones_r")
            nc.vector.tensor_copy(ones_r[:], ones_f[:])
            ones_bf = const_p.tile([128, 1], bf16, tag="ones_bf")
            nc.gpsimd.memset(ones_bf[:], 1.0)

            # resident loads
            wk_sb = wkv_p.tile([128, NIT * KVD], bf16, tag="wk")
            nc.sync.dma_start(
                wk_sb[:].rearrange("p (i d) -> p i d", i=NIT),
                wk.ap().rearrange("(i p) d -> p i d", p=128))
            wv_sb = wkv_p.tile([128, NIT * KVD], bf16, tag="wv")
            nc.sync.dma_start(
                wv_sb[:].rearrange("p (i d) -> p i d", i=NIT),
                wv.ap().rearrange("(i p) d -> p i d", p=128))
            cos_sb = res_p.tile([D, S], bf16, tag="cos")
            nc.sync.dma_start(cos_sb[:], cosT.ap())
            sin_sb = res_p.tile([D, S], bf16, tag="sin")
            nc.sync.dma_start(sin_sb[:], sinS.ap())
            dm_sb = res_p.tile([KB, max(nmask, 1) * CH], bf16, tag="dm")
            nc.sync.dma_start(
                dm_sb[:].rearrange("p (m s) -> p m s", m=max(nmask, 1)),
                dmask.ap().rearrange("m p s -> p m s"))
            kT_sb = [res_p.tile([D, S], bf16, tag=f"kT{kj}", name=f"kT{kj}")
                     for kj in range(KVH)]
            v_sb = res_p.tile([128, (S // 128) * KVD], bf16, tag="v")

            Exp = mybir.ActivationFunctionType.Exp

            def rope(dst, ps, c):
                """dst (bf16 [128,CH]) = rope(ps) with cos/sin chunk c."""
                if stages == "proj_norope":
                    nc.vector.tensor_copy(dst, ps[:])
                    return
                cs = cos_sb[:, c * CH:(c + 1) * CH]
                sn = sin_sb[:, c * CH:(c + 1) * CH]
                m1 = rope_p.tile([128, CH], fp32, tag="m1")
                m2 = rope_p.tile([128, CH], fp32, tag="m2")
                nc.vector.tensor_mul(m1[:], ps[:], cs)
                nc.vector.tensor_mul(m2[0:64, :], ps[64:128, :], sn[0:64, :])
                nc.vector.tensor_mul(m2[64:128, :], ps[0:64, :], sn[64:128, :])
                nc.vector.tensor_add(dst, m1[:], m2[:])

            def proj(c, qt):
                xt = xt_p.tile([128, NIT * CH], bf16, tag="xt")
                nc.sync.dma_start(
                    xt[:].rearrange("p (i s) -> p i s", i=NIT),
                    xT.ap()[:, c * CH:(c + 1) * CH]
                        .rearrange("(i p) s -> p i s", p=128))
                for j in range(QH):
                    wq_t = wq_p.tile([128, NIT * 128], bf16, tag="wq")
                    nc.sync.dma_start(
                        wq_t[:].rearrange("p (i d) -> p i d", i=NIT),
                        wq.ap()[:, j * 128:(j + 1) * 128]
                            .rearrange("(i p) d -> p i d", p=128))
                    ps = pA.tile([128, CH], fp32, tag="pA")
                    for i in range(NIT):
                        nc.tensor.matmul(
                            ps[:], wq_t[:, i * 128:(i + 1) * 128],
                            xt[:, i * CH:(i + 1) * CH],
                            start=(i == 0), stop=(i == NIT - 1))
                    rope(qt[:, j * CH:(j + 1) * CH], ps, c)
                for kj in range(KVH):
                    ps = pA.tile([128, CH], fp32, tag="pA")
                    for i in range(NIT):
                        nc.tensor.matmul(
                            ps[:], wk_sb[:, i * KVD + kj * 128:
                                         i * KVD + (kj + 1) * 128],
                            xt[:, i * CH:(i + 1) * CH],
                            start=(i == 0), stop=(i == NIT - 1))
                    rope(kT_sb[kj][:, c * CH:(c + 1) * CH], ps, c)
                for t in range(CH // 128):
                    ps = pA.tile([128, KVD], fp32, tag="pA")
                    for i in range(NIT):
                        nc.tensor.matmul(
                            ps[:], xt[:, i * CH + t * 128:i * CH + (t + 1) * 128],
                            wv_sb[:, i * KVD:(i + 1) * KVD],
                            start=(i == 0), stop=(i == NIT - 1))
                    sb = (c * (CH // 128) + t) * KVD
                    nc.scalar.copy(v_sb[:, sb:sb + KVD], ps[:])

            def attn(c, qt):
                blocks = plan[c]
                nb = len(blocks)
                for h in range(QH):
                    kvh = h // (QH // KVH)
                    qs = qt[:, h * CH:(h + 1) * CH]
                    po_t = po.tile([128, CH], fp32, tag="po")
                    su_t = psums.tile([1, CH], fp32, tag="su")
                    # software pipeline: scores one block ahead of PV
                    sc_tiles = {}

                    def emit_sc(bi):
                        t, _ = blocks[bi]
                        p = psc.tile([128, CH], fp32, tag="psc")
                        nc.tensor.matmul(
                            p[:], kT_sb[kvh][:, t * KB:(t + 1) * KB], qs,
                            start=True, stop=True, skip_group_check=True)
                        sc_tiles[bi] = p

                    emit_sc(0)
                    for bi in range(nb):
                        if bi + 1 < nb:
                            emit_sc(bi + 1)
                        t, mi = blocks[bi]
                        p = sc_tiles.pop(bi)
                        pr = probs_p.tile([KB, CH], bf16, tag="pr")
                        nc.scalar.activation(pr[:], p[:], Exp, scale=SM_SCALE)
                        if mi is not None:
                            nc.vector.tensor_mul(
                                pr[:], pr[:], dm_sb[:, mi * CH:(mi + 1) * CH])
                        first, last = (bi == 0), (bi == nb - 1)
                        vsl = v_sb[:, t * KVD + kvh * 128:t * KVD + (kvh + 1) * 128]
                        nc.tensor.matmul(po_t[:], vsl, pr[:], start=first,
                                         stop=last, skip_group_check=True)
                        nc.tensor.matmul(su_t[:], ones_bf[:], pr[:], start=first,
                                         stop=last, skip_group_check=True)
                    rc = recip_p.tile([1, CH], fp32, tag="rc")
                    nc.vector.reciprocal(rc[:], su_t[:])
                    rr = recip_p.tile([1, CH], f32r, tag="rr")
                    nc.vector.tensor_copy(rr[:], rc[:])
                    pb = pbc.tile([128, CH], fp32, tag="pb")
                    nc.tensor.matmul(pb[:], ones_r[:], rr[:], start=True,
                                     stop=True, skip_group_check=True)
                    pbs = recip_p.tile([128, CH], fp32, tag="pbs")
                    nc.scalar.copy(pbs[:], pb[:])
                    at = attn_p.tile([128, CH], bf16, tag="at")
                    nc.vector.tensor_mul(at[:], po_t[:], pbs[:])
                    nc.sync.dma_start(bnc_in[c].ap()[h * 128:(h + 1) * 128, :],
                                      at[:])
                nc.gpsimd.collective_compute(
                    "AllGather", mybir.AluOpType.bypass,
                    replica_groups=REPLICA_GROUPS,
                    ins=[bnc_in[c].ap().opt()], outs=[bnc_out[c].ap().opt()])

            def wo_phase(c):
                gt = gath_p.tile([128, NIT * CH], bf16, tag="gt")
                nc.sync.dma_start(
                    gt[:].rearrange("p (i s) -> p i s", i=NIT),
                    bnc_out[c].ap().rearrange("(i p) s -> p i s", p=128))
                for oj in range(QH):
                    wo_t = wo_p.tile([128, NIT * 128], bf16, tag="wo")
                    nc.sync.dma_start(
                        wo_t[:].rearrange("p (i d) -> p i d", i=NIT),
                        wo.ap()[:, oj * 128:(oj + 1) * 128]
                            .rearrange("(i p) d -> p i d", p=128))
                    ps = pA.tile([128, CH], fp32, tag="pA")
                    for i in range(NIT):
                        nc.tensor.matmul(
                            ps[:], wo_t[:, i * 128:(i + 1) * 128],
                            gt[:, i * CH:(i + 1) * CH],
                            start=(i == 0), stop=(i == NIT - 1))
                    ot = out_p.tile([128, CH], fp32, tag="ot")
                    nc.scalar.copy(ot[:], ps[:])
                    nc.sync.dma_start(
                        outT.ap()[oj * 128:(oj + 1) * 128, c * CH:(c + 1) * CH],
                        ot[:])

            # pipeline: wo(c) emitted after attn(c+1) so the all-gather hides
            qts = {}
            for c in range(NCH):
                qts[c] = qt_p.tile([128, QH * CH], bf16, tag="qt", name=f"qt{c}")
                proj(c, qts[c])
                if debug_dumps and c == 0:
                    nc.sync.dma_start(dbg["qt0"].ap(), qts[c][:])
                if stages == "all":
                    attn(c, qts.pop(c))
                    if c >= 1:
                        wo_phase(c - 1)
                else:
                    qts.pop(c)
            if stages == "all":
                wo_phase(NCH - 1)
            if debug_dumps:
                nc.sync.dma_start(dbg["kt0"].ap(), kT_sb[0][:])
                nc.sync.dma_start(dbg["v"].ap(), v_sb[:])
                nc.sync.dma_start(dbg["wk"].ap(), wk_sb[:])
                nc.sync.dma_start(dbg["wv"].ap(), wv_sb[:])
                nc.sync.dma_start(dbg["dm"].ap(), dm_sb[:, 0:4 * CH])
                if stages == "all":
                    nc.sync.dma_start(dbg["bin0"].ap(), bnc_in[0].ap())
                    nc.sync.dma_start(dbg["bout0"].ap(), bnc_out[0].ap())
                else:
                    z = out_p.tile([128, CH], fp32, tag="z", name="z")
                    nc.gpsimd.memset(z[:], 0.0)
                    for oj in range(QH):
                        for cc in range(NCH):
                            nc.sync.dma_start(
                                outT.ap()[oj * 128:(oj + 1) * 128,
                                          cc * CH:(cc + 1) * CH], z[:])

    nc.compile()
    return nc


# ---------------------------------------------------------------- host side
def _rope_cache():
    fi = np.arange(0, D, 2, dtype=np.float32)
    inv = 1.0 / 10000.0 ** (fi / D)
    ang = np.outer(np.arange(S, dtype=np.float32), inv)  # (S, 64)
    cos = np.concatenate([np.cos(ang)] * 2, -1)          # (S, 128)
    sin = np.sin(ang)
    sinS = np.concatenate([-sin, sin], -1)               # signed
    return (np.ascontiguousarray(cos.T).astype(BF16),
            np.ascontiguousarray(sinS.T).astype(BF16))


def _plan_from_mask(mask):
    """Returns (plan, dmask_per_batch[b] -> np[nm,128,CH] bf16)."""
    m = np.asarray(mask[:, 0])                    # (B, S, S) bool, [q, k]
    tril = np.tril(np.ones((S, S), bool))
    if all(np.array_equal(m[b], tril) for b in range(B)):
        plan = []
        for c in range(NCH):
            blk = [(t, None) for t in range(4 * c)]
            blk += [(4 * c + tt, tt) for tt in range(4)]
            plan.append(blk)
        dm = np.zeros((4, KB, CH), np.float32)
        for tt in range(4):
            for p in range(KB):
                dm[tt, p, tt * KB + p:] = 1.0
        dms = [dm.astype(BF16)] * B
        return plan, dms
    if m.all():
        plan = [[(t, None) for t in range(S // KB)] for _ in range(NCH)]
        z = np.zeros((1, KB, CH), BF16)
        return plan, [z] * B
    # generic: classify blocks against the union across batches
    mT = [np.ascontiguousarray(m[b].T) for b in range(B)]  # [k, q]
    plan, tiles = [], [[] for _ in range(B)]
    nm = 0
    for c in range(NCH):
        blk = []
        for t in range(S // KB):
            subs = [mT[b][t * KB:(t + 1) * KB, c * CH:(c + 1) * CH]
                    for b in range(B)]
            if all(not s.any() for s in subs):
                continue
            if all(s.all() for s in subs):
                blk.append((t, None))
            else:
                blk.append((t, nm))
                for b in range(B):
                    tiles[b].append(subs[b].astype(BF16))
                nm += 1
        plan.append(blk)
    dms = [np.stack(tiles[b]) if nm else np.zeros((1, KB, CH), BF16)
           for b in range(B)]
    return plan, dms


def _prep_inputs(x, mask, Wq, Wk, Wv, Wo):
    cosT, sinS = _rope_cache()
    plan, dms = _plan_from_mask(mask)
    in_maps = []
    for c in range(NCORES):
        b, g = c // GRP, c % GRP
        in_maps.append({
            "xT": np.ascontiguousarray(x[b].T).astype(BF16),
            "wq": np.ascontiguousarray(Wq[:, g * QD:(g + 1) * QD]).astype(BF16),
            "wk": np.ascontiguousarray(Wk[:, g * KVD:(g + 1) * KVD]).astype(BF16),
            "wv": np.ascontiguousarray(Wv[:, g * KVD:(g + 1) * KVD]).astype(BF16),
            "wo": np.ascontiguousarray(Wo[:, g * QD:(g + 1) * QD]).astype(BF16),
            "cosT": cosT,
            "sinS": sinS,
            "dmask": np.ascontiguousarray(dms[b]),
        })
    return plan, in_maps


def _get_nc(plan, nmask, debug_dumps=False, stages="all"):
    key = (tuple(tuple(blk) for blk in plan), nmask, debug_dumps, stages)
    if key not in _CACHE:
        _CACHE[key] = build_nc(plan, nmask, debug_dumps, stages)
    return _CACHE[key]


def run(x, mask, Wq, Wk, Wv, Wo, trace=False, debug_dumps=False, stages="all"):
    plan, in_maps = _prep_inputs(x, mask, Wq, Wk, Wv, Wo)
    nc = _get_nc(plan, in_maps[0]["dmask"].shape[0], debug_dumps, stages)
    res = bass_utils.run_bass_kernel_spmd(
        nc, in_maps, core_ids=list(range(NCORES)), trace=trace)
    out = np.empty((B, S, HID), np.float32)
    for c in range(NCORES):
        b, g = c // GRP, c % GRP
        out[b, :, g * QD:(g + 1) * QD] = res.results[c]["outT"].T
    return out, res


def kernel(x, mask, Wq, Wk, Wv, Wo):
    out, _ = run(np.asarray(x), np.asarray(mask), np.asarray(Wq),
                 np.asarray(Wk), np.asarray(Wv), np.asarray(Wo))
    return out


# needed only when profiling (trace=True) inside this container
def install_ntff_hook():
    try:
        from antenv.axon_hooks import get_axon_ntff_profile_hook  # noqa: F401
        return
    except ImportError:
        pass
    import types
    import antenv
    try:
        from trn_agent_boot.trn_boot import _ntff_profile_via_ctypes
        hook = _ntff_profile_via_ctypes('/opt/axon/libaxon_pjrt.so')
    except Exception:
        hook = None
    mod = types.ModuleType("antenv.axon_hooks")
    state = {"h": hook}
    mod.get_axon_ntff_profile_hook = lambda: state["h"]
    mod.set_axon_ntff_profile_hook = lambda h: state.__setitem__("h", h)
    sys.modules["antenv.axon_hooks"] = mod
    antenv.axon_hooks = mod


install_ntff_hook()
bass_utils.upload_artifacts = lambda tmpdir: "local://" + str(tmpdir)


# revision 11
# speedup vs baseline: 1.1165x; 1.1165x over previous
"""Trainium2 Bass kernel for GQA attention (dense_transformer).

Sharding: 8 cores = 2-way data parallel (batch) x 4-way tensor parallel (heads).
Core c handles batch b=c//4, head group g=c%4 (8 q heads, 2 kv heads).
Per core: QKV projections (bf16, f32 accum), RoPE, causal attention with
transposed-scores layout (scoresT[k,q] -> probsT used directly as the moving
operand of the PV matmul; no on-chip transposes), per-chunk AllGather of
attention outputs within each 4-core group, then a column-sharded Wo matmul.
Host assembles disjoint output shards (pure unshard, no host math).

All inputs are host-packed into the exact SBUF tile layouts so every DMA is a
plain 2D transfer (contiguous per partition).
"""
import sys

sys.path.insert(0, "/opt/trn_rl_repo")

import numpy as np
import ml_dtypes

import concourse.bacc as bacc
import concourse.mybir as mybir
import concourse.tile as tile
from concourse import bass_utils

BF16 = ml_dtypes.bfloat16

B, S, HID = 2, 2048, 4096
NH, NKV, D = 32, 8, 128
NCORES, GRP = 8, 4          # 2 groups of 4 cores
QH, KVH = NH // GRP, NKV // GRP   # 8 q heads, 2 kv heads per core
QD, KVD = QH * D, KVH * D         # 1024, 256
CH, NCH = 512, S // 512           # q-chunk size / count
KB = 128                          # k block
NIT = HID // 128                  # 32 contraction tiles
SM_SCALE = float(D) ** -0.5
REPLICA_GROUPS = [[0, 1, 2, 3], [4, 5, 6, 7]]

_CACHE: dict = {}


# ---------------------------------------------------------------- builder
def build_nc(plan, nmask, debug_dumps=False):
    """plan[c] = list of (t, mask_idx_or_None) k-blocks for q-chunk c."""
    fp32, bf16, f32r = mybir.dt.float32, mybir.dt.bfloat16, mybir.dt.float32r
    nc = bacc.Bacc("TRN2", target_bir_lowering=False, debug=False,
                   num_devices=NCORES)

    # host-packed inputs (exact SBUF layouts; all DMAs contiguous/partition)
    xT = nc.dram_tensor("xT", [NCH, 128, NIT * CH], bf16, kind="ExternalInput")
    wq = nc.dram_tensor("wq", [QH, 128, NIT * 128], bf16, kind="ExternalInput")
    wk = nc.dram_tensor("wk", [128, NIT * KVD], bf16, kind="ExternalInput")
    wv = nc.dram_tensor("wv", [128, NIT * KVD], bf16, kind="ExternalInput")
    wo = nc.dram_tensor("wo", [QH, 128, NIT * 128], bf16, kind="ExternalInput")
    cosT = nc.dram_tensor("cosT", [D, S], bf16, kind="ExternalInput")
    sinS = nc.dram_tensor("sinS", [D, S], bf16, kind="ExternalInput")
    nm = max(nmask, 1)
    dmask = nc.dram_tensor("dmask", [KB, nm * CH], bf16, kind="ExternalInput")
    outT = nc.dram_tensor("outT", [QD, S], fp32, kind="ExternalOutput")

    # partition-major bounce buffers: bnc_in[p, h*CH+s]; gather concatenates
    # the 4 group members along dim0 -> [GRP*128, QH*CH]
    bnc_in = [nc.dram_tensor(f"bnc_in{c}", [128, QH * CH], bf16)
              for c in range(NCH)]
    bnc_out = [nc.dram_tensor(f"bnc_out{c}", [GRP * 128, QH * CH], bf16)
               for c in range(NCH)]

    dbg = {}
    if debug_dumps:
        dbg["qt0"] = nc.dram_tensor("dbg_qt0", [128, QH * CH], bf16,
                                    kind="ExternalOutput")
        dbg["kt0"] = nc.dram_tensor("dbg_kt0", [D, S], bf16,
                                    kind="ExternalOutput")
        dbg["v"] = nc.dram_tensor("dbg_v", [128, (S // 128) * KVD], bf16,
                                  kind="ExternalOutput")

    with tile.TileContext(nc) as tc:
        from contextlib import ExitStack
        with ExitStack() as ctx:
            P = lambda **kw: ctx.enter_context(tc.tile_pool(**kw))
            const_p = P(name="const", bufs=1)
            wkv_p = P(name="wkv", bufs=1)
            res_p = P(name="res", bufs=1)         # kT, v, cos, sin, masks
            xt_p = P(name="xt", bufs=1)
            wq_p = P(name="wqp", bufs=2)
            qt_p = P(name="qt", bufs=2)
            rope_p = P(name="rope", bufs=2)
            probs_p = P(name="probs", bufs=4)
            attn_p = P(name="attn", bufs=3)
            gath_p = P(name="gath", bufs=1)
            wo_p = P(name="wop", bufs=2)
            out_p = P(name="outp", bufs=2)
            recip_p = P(name="recip", bufs=2)
            pA = P(name="pA", bufs=2, space="PSUM")
            psc = P(name="psc", bufs=2, space="PSUM")
            po = P(name="po", bufs=2, space="PSUM")
            psums = P(name="psums", bufs=1, space="PSUM")
            pbc = P(name="pbc", bufs=1, space="PSUM")

            # constants
            ones_f = const_p.tile([1, 128], fp32, tag="ones_f")
            nc.gpsimd.memset(ones_f[:], 1.0)
            ones_r = const_p.tile([1, 128], f32r, tag="ones_r")
            nc.vector.tensor_copy(ones_r[:], ones_f[:])
            ones_bf = const_p.tile([128, 1], bf16, tag="ones_bf")
            nc.gpsimd.memset(ones_bf[:], 1.0)

            # resident loads (all straight 2D)
            wk_sb = wkv_p.tile([128, NIT * KVD], bf16, tag="wk")
            nc.sync.dma_start(wk_sb[:], wk.ap())
            wv_sb = wkv_p.tile([128, NIT * KVD], bf16, tag="wv")
            nc.sync.dma_start(wv_sb[:], wv.ap())
            cos_sb = res_p.tile([D, S], bf16, tag="cos")
            nc.sync.dma_start(cos_sb[:], cosT.ap())
            sin_sb = res_p.tile([D, S], bf16, tag="sin")
            nc.sync.dma_start(sin_sb[:], sinS.ap())
            dm_sb = res_p.tile([KB, nm * CH], bf16, tag="dm")
            nc.sync.dma_start(dm_sb[:], dmask.ap())
            kT_sb = [res_p.tile([D, S], bf16, tag=f"kT{kj}", name=f"kT{kj}")
                     for kj in range(KVH)]
            v_sb = res_p.tile([128, (S // 128) * KVD], bf16, tag="v")

            Exp = mybir.ActivationFunctionType.Exp

            def rope(dst, ps, c):
                """dst (bf16 [128,CH]) = rope(ps) with cos/sin chunk c."""
                cs = cos_sb[:, c * CH:(c + 1) * CH]
                sn = sin_sb[:, c * CH:(c + 1) * CH]
                m1 = rope_p.tile([128, CH], fp32, tag="m1")
                m2 = rope_p.tile([128, CH], fp32, tag="m2")
                nc.vector.tensor_mul(m1[:], ps[:], cs)
                nc.vector.tensor_mul(m2[0:64, :], ps[64:128, :], sn[0:64, :])
                nc.vector.tensor_mul(m2[64:128, :], ps[0:64, :], sn[64:128, :])
                nc.vector.tensor_add(dst, m1[:], m2[:])

            def proj(c, qt):
                xt = xt_p.tile([128, NIT * CH], bf16, tag="xt")
                nc.sync.dma_start(xt[:], xT.ap()[c])
                for j in range(QH):
                    wq_t = wq_p.tile([128, NIT * 128], bf16, tag="wq")
                    nc.sync.dma_start(wq_t[:], wq.ap()[j])
                    ps = pA.tile([128, CH], fp32, tag="pA")
                    for i in range(NIT):
                        nc.tensor.matmul(
                            ps[:], wq_t[:, i * 128:(i + 1) * 128],
                            xt[:, i * CH:(i + 1) * CH],
                            start=(i == 0), stop=(i == NIT - 1))
                    rope(qt[:, j * CH:(j + 1) * CH], ps, c)
                for kj in range(KVH):
                    ps = pA.tile([128, CH], fp32, tag="pA")
                    for i in range(NIT):
                        nc.tensor.matmul(
                            ps[:], wk_sb[:, i * KVD + kj * 128:
                                         i * KVD + (kj + 1) * 128],
                            xt[:, i * CH:(i + 1) * CH],
                            start=(i == 0), stop=(i == NIT - 1))
                    rope(kT_sb[kj][:, c * CH:(c + 1) * CH], ps, c)
                for t in range(CH // 128):
                    ps = pA.tile([128, KVD], fp32, tag="pA")
                    for i in range(NIT):
                        nc.tensor.matmul(
                            ps[:], xt[:, i * CH + t * 128:i * CH + (t + 1) * 128],
                            wv_sb[:, i * KVD:(i + 1) * KVD],
                            start=(i == 0), stop=(i == NIT - 1))
                    sb = (c * (CH // 128) + t) * KVD
                    nc.scalar.copy(v_sb[:, sb:sb + KVD], ps[:])

            # deferred normalization tail: after the last ones-matmul of head
            # h, only the (cheap) reciprocal is emitted inline; the broadcast
            # matmul + final multiply + DMA run while head h+1's scores are
            # already streaming on PE.
            def make_tail(c, h, po_t, su_t):
                rc = recip_p.tile([1, CH], fp32, tag="rc", name=f"rc{c}_{h}")
                nc.vector.reciprocal_approx_fast(rc[:], su_t[:])
                rr = recip_p.tile([1, CH], f32r, tag="rr", name=f"rr{c}_{h}")
                nc.vector.tensor_copy(rr[:], rc[:])

                def flush():
                    pb = pbc.tile([128, CH], fp32, tag="pb")
                    nc.tensor.matmul(pb[:], ones_r[:], rr[:], start=True,
                                     stop=True, skip_group_check=True)
                    pbs = recip_p.tile([128, CH], fp32, tag="pbs")
                    nc.scalar.copy(pbs[:], pb[:])
                    at = attn_p.tile([128, CH], bf16, tag="at")
                    nc.vector.tensor_mul(at[:], po_t[:], pbs[:])
                    nc.scalar.dma_start(
                        bnc_in[c].ap()[:, h * CH:(h + 1) * CH], at[:])
                return flush

            def attn(c, qt):
                blocks = plan[c]
                nb = len(blocks)
                pending = None
                for h in range(QH):
                    kvh = h // (QH // KVH)
                    qs = qt[:, h * CH:(h + 1) * CH]
                    po_t = po.tile([128, CH], fp32, tag="po")
                    su_t = psums.tile([1, CH], fp32, tag="su")
                    sc_tiles = {}

                    def emit_sc(bi):
                        t, _ = blocks[bi]
                        p = psc.tile([128, CH], fp32, tag="psc")
                        nc.tensor.matmul(
                            p[:], kT_sb[kvh][:, t * KB:(t + 1) * KB], qs,
                            start=True, stop=True, skip_group_check=True)
                        sc_tiles[bi] = p

                    emit_sc(0)
                    for bi in range(nb):
                        if bi + 1 < nb:
                            emit_sc(bi + 1)
                        if bi == 1 and pending is not None:
                            pending()
                            pending = None
                        t, mi = blocks[bi]
                        p = sc_tiles.pop(bi)
                        pr = probs_p.tile([KB, CH], bf16, tag="pr")
                        nc.scalar.activation(pr[:], p[:], Exp, scale=SM_SCALE)
                        if mi is not None:
                            nc.vector.tensor_mul(
                                pr[:], pr[:], dm_sb[:, mi * CH:(mi + 1) * CH])
                        first, last = (bi == 0), (bi == nb - 1)
                        vsl = v_sb[:, t * KVD + kvh * 128:t * KVD + (kvh + 1) * 128]
                        nc.tensor.matmul(po_t[:], vsl, pr[:], start=first,
                                         stop=last, skip_group_check=True)
                        nc.tensor.matmul(su_t[:], ones_bf[:], pr[:], start=first,
                                         stop=last, skip_group_check=True)
                    if pending is not None:
                        pending()
                    pending = make_tail(c, h, po_t, su_t)
                pending()
                nc.gpsimd.collective_compute(
                    "AllGather", mybir.AluOpType.bypass,
                    replica_groups=REPLICA_GROUPS,
                    ins=[bnc_in[c].ap().opt()], outs=[bnc_out[c].ap().opt()])

            def wo_phase(c):
                gt = gath_p.tile([128, GRP * QH * CH], bf16, tag="gt")
                nc.scalar.dma_start(
                    gt[:].rearrange("p (g f) -> p g f", g=GRP),
                    bnc_out[c].ap().rearrange("(g p) f -> p g f", p=128))
                for oj in range(QH):
                    wo_t = wo_p.tile([128, NIT * 128], bf16, tag="wo")
                    nc.sync.dma_start(wo_t[:], wo.ap()[oj])
                    ps = pA.tile([128, CH], fp32, tag="pA")
                    for i in range(NIT):
                        nc.tensor.matmul(
                            ps[:], wo_t[:, i * 128:(i + 1) * 128],
                            gt[:, i * CH:(i + 1) * CH],
                            start=(i == 0), stop=(i == NIT - 1))
                    ot = out_p.tile([128, CH], fp32, tag="ot")
                    nc.scalar.copy(ot[:], ps[:])
                    nc.scalar.dma_start(
                        outT.ap()[oj * 128:(oj + 1) * 128, c * CH:(c + 1) * CH],
                        ot[:])

            # pipeline: wo(c) emitted after attn(c+1) so the all-gather hides
            qts = {}
            for c in range(NCH):
                qts[c] = qt_p.tile([128, QH * CH], bf16, tag="qt", name=f"qt{c}")
                proj(c, qts[c])
                if debug_dumps and c == 0:
                    nc.sync.dma_start(dbg["qt0"].ap(), qts[c][:])
                attn(c, qts.pop(c))
                if c >= 1:
                    wo_phase(c - 1)
            wo_phase(NCH - 1)
            if debug_dumps:
                nc.sync.dma_start(dbg["kt0"].ap(), kT_sb[0][:])
                nc.sync.dma_start(dbg["v"].ap(), v_sb[:])

    nc.compile()
    return nc


# ---------------------------------------------------------------- host side
def _rope_cache():
    fi = np.arange(0, D, 2, dtype=np.float32)
    inv = 1.0 / 10000.0 ** (fi / D)
    ang = np.outer(np.arange(S, dtype=np.float32), inv)  # (S, 64)
    cos = np.concatenate([np.cos(ang)] * 2, -1)          # (S, 128)
    sin = np.sin(ang)
    sinS = np.concatenate([-sin, sin], -1)               # signed
    return (np.ascontiguousarray(cos.T).astype(BF16),
            np.ascontiguousarray(sinS.T).astype(BF16))


def _plan_from_mask(mask):
    """Returns (plan, dmask_per_batch[b] -> np[nm,128,CH] bf16)."""
    m = np.asarray(mask[:, 0])                    # (B, S, S) bool, [q, k]
    tril = np.tril(np.ones((S, S), bool))
    if all(np.array_equal(m[b], tril) for b in range(B)):
        plan = []
        for c in range(NCH):
            blk = [(t, None) for t in range(4 * c)]
            blk += [(4 * c + tt, tt) for tt in range(4)]
            plan.append(blk)
        dm = np.zeros((4, KB, CH), np.float32)
        for tt in range(4):
            for p in range(KB):
                dm[tt, p, tt * KB + p:] = 1.0
        dms = [dm.astype(BF16)] * B
        return plan, dms
    if m.all():
        plan = [[(t, None) for t in range(S // KB)] for _ in range(NCH)]
        z = np.zeros((1, KB, CH), BF16)
        return plan, [z] * B
    # generic: classify blocks against the union across batches
    mT = [np.ascontiguousarray(m[b].T) for b in range(B)]  # [k, q]
    plan, tiles = [], [[] for _ in range(B)]
    nm = 0
    for c in range(NCH):
        blk = []
        for t in range(S // KB):
            subs = [mT[b][t * KB:(t + 1) * KB, c * CH:(c + 1) * CH]
                    for b in range(B)]
            if all(not s.any() for s in subs):
                continue
            if all(s.all() for s in subs):
                blk.append((t, None))
            else:
                blk.append((t, nm))
                for b in range(B):
                    tiles[b].append(subs[b].astype(BF16))
                nm += 1
        plan.append(blk)
    dms = [np.stack(tiles[b]) if nm else np.zeros((1, KB, CH), BF16)
           for b in range(B)]
    return plan, dms


def _pack_ip(w, nj):
    """[HID, nj*d] -> [nj, 128, NIT*d] tile-packed (i along free)."""
    hid, cols = w.shape
    d = cols // nj
    r = w.reshape(NIT, 128, nj, d).transpose(2, 1, 0, 3)
    return np.ascontiguousarray(r.reshape(nj, 128, NIT * d))


def _prep_inputs(x, mask, Wq, Wk, Wv, Wo):
    cosT, sinS = _rope_cache()
    plan, dms = _plan_from_mask(mask)
    dms_packed = []
    for b in range(B):
        dm = dms[b]  # [nm, 128, 512]
        dms_packed.append(np.ascontiguousarray(
            dm.transpose(1, 0, 2).reshape(KB, -1)))
    xp = {}
    for b in range(B):
        xb = np.ascontiguousarray(x[b].T).astype(BF16)      # [HID, S]
        r = xb.reshape(NIT, 128, NCH, CH).transpose(2, 1, 0, 3)
        xp[b] = np.ascontiguousarray(r.reshape(NCH, 128, NIT * CH))
    in_maps = []
    for c in range(NCORES):
        b, g = c // GRP, c % GRP
        wq_g = Wq[:, g * QD:(g + 1) * QD].astype(BF16)
        wk_g = Wk[:, g * KVD:(g + 1) * KVD].astype(BF16)
        wv_g = Wv[:, g * KVD:(g + 1) * KVD].astype(BF16)
        wo_g = Wo[:, g * QD:(g + 1) * QD].astype(BF16)
        in_maps.append({
            "xT": xp[b],
            "wq": _pack_ip(wq_g, QH),
            "wk": _pack_ip(wk_g, 1)[0],
            "wv": _pack_ip(wv_g, 1)[0],
            "wo": _pack_ip(wo_g, QH),
            "cosT": cosT,
            "sinS": sinS,
            "dmask": dms_packed[b],
        })
    return plan, in_maps


def _get_nc(plan, nmask, debug_dumps=False):
    key = (tuple(tuple(blk) for blk in plan), nmask, debug_dumps)
    if key not in _CACHE:
        _CACHE[key] = build_nc(plan, nmask, debug_dumps)
    return _CACHE[key]


def run(x, mask, Wq, Wk, Wv, Wo, trace=False, debug_dumps=False):
    plan, in_maps = _prep_inputs(x, mask, Wq, Wk, Wv, Wo)
    nmask = in_maps[0]["dmask"].shape[1] // CH
    nc = _get_nc(plan, nmask, debug_dumps)
    res = bass_utils.run_bass_kernel_spmd(
        nc, in_maps, core_ids=list(range(NCORES)), trace=trace)
    out = np.empty((B, S, HID), np.float32)
    for c in range(NCORES):
        b, g = c // GRP, c % GRP
        out[b, :, g * QD:(g + 1) * QD] = res.results[c]["outT"].T
    return out, res


def kernel(x, mask, Wq, Wk, Wv, Wo):
    out, _ = run(np.asarray(x), np.asarray(mask), np.asarray(Wq),
                 np.asarray(Wk), np.asarray(Wv), np.asarray(Wo))
    return out


# needed only when profiling (trace=True) inside this container
def install_ntff_hook():
    try:
        from antenv.axon_hooks import get_axon_ntff_profile_hook  # noqa: F401
        return
    except ImportError:
        pass
    import types
    import antenv
    try:
        from trn_agent_boot.trn_boot import _ntff_profile_via_ctypes
        hook = _ntff_profile_via_ctypes('/opt/axon/libaxon_pjrt.so')
    except Exception:
        hook = None
    mod = types.ModuleType("antenv.axon_hooks")
    state = {"h": hook}
    mod.get_axon_ntff_profile_hook = lambda: state["h"]
    mod.set_axon_ntff_profile_hook = lambda h: state.__setitem__("h", h)
    sys.modules["antenv.axon_hooks"] = mod
    antenv.axon_hooks = mod


install_ntff_hook()
bass_utils.upload_artifacts = lambda tmpdir: "local://" + str(tmpdir)


# revision 17
# speedup vs baseline: 1.2242x; 1.0965x over previous
"""Trainium2 Bass kernel for GQA attention (dense_transformer).

Sharding: 8 cores = 2-way data parallel (batch) x 4-way tensor parallel (heads).
Core c handles batch b=c//4, head group g=c%4 (8 q heads, 2 kv heads).
Per core: QKV projections (bf16, f32 accum), RoPE, causal attention with
transposed-scores layout (scoresT[k,q] -> probsT used directly as the moving
operand of the PV matmul; no on-chip transposes), per-chunk AllGather of
attention outputs within each 4-core group, then a column-sharded Wo matmul.
Host assembles disjoint output shards (pure unshard, no host math).

All inputs are host-packed into the exact SBUF tile layouts so every DMA is a
plain 2D transfer (contiguous per partition).
"""
import sys

sys.path.insert(0, "/opt/trn_rl_repo")

import numpy as np
import ml_dtypes

import concourse.bacc as bacc
import concourse.mybir as mybir
import concourse.tile as tile
from concourse import bass_utils

BF16 = ml_dtypes.bfloat16

B, S, HID = 2, 2048, 4096
NH, NKV, D = 32, 8, 128
NCORES, GRP = 8, 4          # 2 groups of 4 cores
QH, KVH = NH // GRP, NKV // GRP   # 8 q heads, 2 kv heads per core
QD, KVD = QH * D, KVH * D         # 1024, 256
CH, NCH = 512, S // 512           # q-chunk size / count
KB = 128                          # k block
NIT = HID // 128                  # 32 contraction tiles
SM_SCALE = float(D) ** -0.5
REPLICA_GROUPS = [[0, 1, 2, 3], [4, 5, 6, 7]]

_CACHE: dict = {}


# ---------------------------------------------------------------- builder
def build_nc(plan, nmask, debug_dumps=False):
    """plan[c] = list of (t, mask_idx_or_None) k-blocks for q-chunk c."""
    fp32, bf16, f32r = mybir.dt.float32, mybir.dt.bfloat16, mybir.dt.float32r
    nc = bacc.Bacc("TRN2", target_bir_lowering=False, debug=False,
                   num_devices=NCORES)

    # host-packed inputs (exact SBUF layouts; all DMAs contiguous/partition)
    xT = nc.dram_tensor("xT", [NCH, 128, NIT * CH], bf16, kind="ExternalInput")
    wq = nc.dram_tensor("wq", [QH, 128, NIT * 128], bf16, kind="ExternalInput")
    wk = nc.dram_tensor("wk", [128, NIT * KVD], bf16, kind="ExternalInput")
    wv = nc.dram_tensor("wv", [128, NIT * KVD], bf16, kind="ExternalInput")
    wo = nc.dram_tensor("wo", [QH, 128, NIT * 128], bf16, kind="ExternalInput")
    cosT = nc.dram_tensor("cosT", [D, S], bf16, kind="ExternalInput")
    sinS = nc.dram_tensor("sinS", [D, S], bf16, kind="ExternalInput")
    nm = max(nmask, 1)
    dmask = nc.dram_tensor("dmask", [KB, nm * CH], bf16, kind="ExternalInput")
    outT = nc.dram_tensor("outT", [QD, S], fp32, kind="ExternalOutput")

    # partition-major bounce buffers: bnc_in[p, h*CH+s]; gather concatenates
    # the 4 group members along dim0 -> [GRP*128, QH*CH]
    bnc_in = [nc.dram_tensor(f"bnc_in{c}", [128, QH * CH], bf16)
              for c in range(NCH)]
    bnc_out = [nc.dram_tensor(f"bnc_out{c}", [GRP * 128, QH * CH], bf16)
               for c in range(NCH)]

    dbg = {}
    if debug_dumps:
        dbg["qt0"] = nc.dram_tensor("dbg_qt0", [128, QH * CH], bf16,
                                    kind="ExternalOutput")
        dbg["kt0"] = nc.dram_tensor("dbg_kt0", [D, S], bf16,
                                    kind="ExternalOutput")
        dbg["v"] = nc.dram_tensor("dbg_v", [128, (S // 128) * KVD], bf16,
                                  kind="ExternalOutput")

    with tile.TileContext(nc) as tc:
        from contextlib import ExitStack
        with ExitStack() as ctx:
            P = lambda **kw: ctx.enter_context(tc.tile_pool(**kw))
            const_p = P(name="const", bufs=1)
            wkv_p = P(name="wkv", bufs=1)
            res_p = P(name="res", bufs=1)         # kT, v, cos, sin, masks
            xt_p = P(name="xt", bufs=1)
            wq_p = P(name="wqp", bufs=2)
            qt_p = P(name="qt", bufs=2)
            rope_p = P(name="rope", bufs=2)
            probs_p = P(name="probs", bufs=3)
            attn_p = P(name="attn", bufs=3)
            gath_p = P(name="gath", bufs=1)
            wo_p = P(name="wop", bufs=2)
            out_p = P(name="outp", bufs=2)
            recip_p = P(name="recip", bufs=2)
            pA = P(name="pA", bufs=2, space="PSUM")
            psc = P(name="psc", bufs=2, space="PSUM")
            po = P(name="po", bufs=2, space="PSUM")
            psums = P(name="psums", bufs=1, space="PSUM")
            pbc = P(name="pbc", bufs=1, space="PSUM")

            # constants
            ones_f = const_p.tile([1, 128], fp32, tag="ones_f")
            nc.gpsimd.memset(ones_f[:], 1.0)
            ones_r = const_p.tile([1, 128], f32r, tag="ones_r")
            nc.vector.tensor_copy(ones_r[:], ones_f[:])
            ones_bf = const_p.tile([128, 1], bf16, tag="ones_bf")
            nc.gpsimd.memset(ones_bf[:], 1.0)
            ones_cf = const_p.tile([128, 1], fp32, tag="ones_cf")
            nc.gpsimd.memset(ones_cf[:], 1.0)
            ones_cr = const_p.tile([128, 1], f32r, tag="ones_cr")
            nc.vector.tensor_copy(ones_cr[:], ones_cf[:])

            # resident loads (all straight 2D)
            wk_sb = wkv_p.tile([128, NIT * KVD], bf16, tag="wk")
            nc.sync.dma_start(wk_sb[:], wk.ap())
            wv_sb = wkv_p.tile([128, NIT * KVD], bf16, tag="wv")
            nc.sync.dma_start(wv_sb[:], wv.ap())
            cos_sb = res_p.tile([D, S], bf16, tag="cos")
            nc.sync.dma_start(cos_sb[:], cosT.ap())
            sin_sb = res_p.tile([D, S], bf16, tag="sin")
            nc.sync.dma_start(sin_sb[:], sinS.ap())
            dm_sb = res_p.tile([KB, nm * CH], bf16, tag="dm")
            nc.sync.dma_start(dm_sb[:], dmask.ap())
            kT_sb = [res_p.tile([D, S], bf16, tag=f"kT{kj}", name=f"kT{kj}")
                     for kj in range(KVH)]
            v_sb = res_p.tile([128, (S // 128) * KVD], bf16, tag="v")

            Exp = mybir.ActivationFunctionType.Exp

            def rope(dst, ps, c):
                """dst (bf16 [128,CH]) = rope(ps) with cos/sin chunk c."""
                cs = cos_sb[:, c * CH:(c + 1) * CH]
                sn = sin_sb[:, c * CH:(c + 1) * CH]
                m1 = rope_p.tile([128, CH], fp32, tag="m1")
                m2 = rope_p.tile([128, CH], fp32, tag="m2")
                nc.vector.tensor_mul(m1[:], ps[:], cs)
                nc.vector.tensor_mul(m2[0:64, :], ps[64:128, :], sn[0:64, :])
                nc.vector.tensor_mul(m2[64:128, :], ps[0:64, :], sn[64:128, :])
                nc.vector.tensor_add(dst, m1[:], m2[:])

            def proj(c, qt):
                xt = xt_p.tile([128, NIT * CH], bf16, tag="xt")
                nc.sync.dma_start(xt[:], xT.ap()[c])
                for j in range(QH):
                    wq_t = wq_p.tile([128, NIT * 128], bf16, tag="wq")
                    nc.sync.dma_start(wq_t[:], wq.ap()[j])
                    ps = pA.tile([128, CH], fp32, tag="pA")
                    for i in range(NIT):
                        nc.tensor.matmul(
                            ps[:], wq_t[:, i * 128:(i + 1) * 128],
                            xt[:, i * CH:(i + 1) * CH],
                            start=(i == 0), stop=(i == NIT - 1))
                    rope(qt[:, j * CH:(j + 1) * CH], ps, c)
                for kj in range(KVH):
                    ps = pA.tile([128, CH], fp32, tag="pA")
                    for i in range(NIT):
                        nc.tensor.matmul(
                            ps[:], wk_sb[:, i * KVD + kj * 128:
                                         i * KVD + (kj + 1) * 128],
                            xt[:, i * CH:(i + 1) * CH],
                            start=(i == 0), stop=(i == NIT - 1))
                    rope(kT_sb[kj][:, c * CH:(c + 1) * CH], ps, c)
                for t in range(CH // 128):
                    ps = pA.tile([128, KVD], fp32, tag="pA")
                    for i in range(NIT):
                        nc.tensor.matmul(
                            ps[:], xt[:, i * CH + t * 128:i * CH + (t + 1) * 128],
                            wv_sb[:, i * KVD:(i + 1) * KVD],
                            start=(i == 0), stop=(i == NIT - 1))
                    sb = (c * (CH // 128) + t) * KVD
                    nc.scalar.copy(v_sb[:, sb:sb + KVD], ps[:])

            # deferred normalization tail: after the last ones-matmul of head
            # h, only the (cheap) reciprocal is emitted inline; the broadcast
            # matmul + final multiply + DMA run while head h+1's scores are
            # already streaming on PE.
            def make_tail(c, h, po_t, su_t):
                rc = recip_p.tile([1, CH], fp32, tag="rc", name=f"rc{c}_{h}")
                nc.vector.reciprocal_approx_fast(rc[:], su_t[:])
                rr = recip_p.tile([1, CH], f32r, tag="rr", name=f"rr{c}_{h}")
                nc.vector.tensor_copy(rr[:], rc[:])

                def flush():
                    pb = pbc.tile([128, CH], fp32, tag="pb")
                    nc.tensor.matmul(pb[:], ones_r[:], rr[:], start=True,
                                     stop=True, skip_group_check=True)
                    pbs = recip_p.tile([128, CH], fp32, tag="pbs")
                    nc.scalar.copy(pbs[:], pb[:])
                    at = attn_p.tile([128, CH], bf16, tag="at")
                    nc.vector.tensor_mul(at[:], po_t[:], pbs[:])
                    nc.scalar.dma_start(
                        bnc_in[c].ap()[:, h * CH:(h + 1) * CH], at[:])
                return flush

            def wo_steps(c):
                """Generator of emission closures for the Wo phase of chunk c
                (interleaved into the next chunk's attention to fill PE
                bubbles left by the exp chain)."""
                gt = gath_p.tile([128, GRP * QH * CH], bf16, tag="gt",
                                 name=f"gt{c}")
                nc.scalar.dma_start(
                    gt[:].rearrange("p (g f) -> p g f", g=GRP),
                    bnc_out[c].ap().rearrange("(g p) f -> p g f", p=128))
                for oj in range(QH):
                    wo_t = wo_p.tile([128, NIT * 128], bf16, tag="wo",
                                     name=f"wo{c}_{oj}")
                    nc.sync.dma_start(wo_t[:], wo.ap()[oj])
                    ps = pA.tile([128, CH], fp32, tag="pA",
                                 name=f"wops{c}_{oj}")
                    for i in range(NIT):
                        def mm(i=i, ps=ps, wo_t=wo_t):
                            nc.tensor.matmul(
                                ps[:], wo_t[:, i * 128:(i + 1) * 128],
                                gt[:, i * CH:(i + 1) * CH],
                                start=(i == 0), stop=(i == NIT - 1),
                                skip_group_check=True)
                        yield mm

                    def fin(ps=ps, oj=oj):
                        ot = out_p.tile([128, CH], fp32, tag="ot")
                        nc.vector.tensor_copy(ot[:], ps[:])
                        nc.scalar.dma_start(
                            outT.ap()[oj * 128:(oj + 1) * 128,
                                      c * CH:(c + 1) * CH], ot[:])
                    yield fin

            def attn(c, qt, wo_iter):
                blocks = plan[c]
                nb = len(blocks)
                # one wo matmul per attention block roughly balances PE
                # against the exp-bound ACT chain; the rest drains after
                wo_per_block = 1
                pending = None
                for h in range(QH):
                    kvh = h // (QH // KVH)
                    qs = qt[:, h * CH:(h + 1) * CH]
                    po_t = po.tile([128, CH], fp32, tag="po")
                    su_t = psums.tile([1, CH], fp32, tag="su")
                    acc = attn_p.tile([128, CH], f32r, tag="acc", bufs=2)
                    sc_tiles = {}

                    def emit_sc(bi):
                        t, _ = blocks[bi]
                        p = psc.tile([128, CH], fp32, tag="psc")
                        nc.tensor.matmul(
                            p[:], kT_sb[kvh][:, t * KB:(t + 1) * KB], qs,
                            start=True, stop=True, skip_group_check=True)
                        sc_tiles[bi] = p

                    emit_sc(0)
                    for bi in range(nb):
                        if bi + 1 < nb:
                            emit_sc(bi + 1)
                        if bi == 1 and pending is not None:
                            pending()
                            pending = None
                        t, mi = blocks[bi]
                        p = sc_tiles.pop(bi)
                        pr = probs_p.tile([KB, CH], bf16, tag="pr")
                        nc.scalar.activation(pr[:], p[:], Exp, scale=SM_SCALE)
                        if mi is not None:
                            nc.vector.tensor_mul(
                                pr[:], pr[:], dm_sb[:, mi * CH:(mi + 1) * CH])
                        first, last = (bi == 0), (bi == nb - 1)
                        vsl = v_sb[:, t * KVD + kvh * 128:t * KVD + (kvh + 1) * 128]
                        nc.tensor.matmul(po_t[:], vsl, pr[:], start=first,
                                         stop=last, skip_group_check=True)
                        if first:
                            nc.vector.tensor_copy(acc[:], pr[:])
                        else:
                            nc.vector.tensor_add(acc[:], acc[:], pr[:])
                        if wo_iter is not None:
                            for _ in range(wo_per_block):
                                step = next(wo_iter, None)
                                if step is None:
                                    wo_iter = None
                                    break
                                step()
                    # single ones-matmul on the accumulated probs
                    nc.tensor.matmul(su_t[:], ones_cr[:], acc[:], start=True,
                                     stop=True, skip_group_check=True)
                    if pending is not None:
                        pending()
                    pending = make_tail(c, h, po_t, su_t)
                pending()
                nc.gpsimd.collective_compute(
                    "AllGather", mybir.AluOpType.bypass,
                    replica_groups=REPLICA_GROUPS,
                    ins=[bnc_in[c].ap().opt()], outs=[bnc_out[c].ap().opt()])
                if wo_iter is not None:
                    for step in wo_iter:
                        step()

            # pipeline: wo(c) interleaved into attn(c+1) block-by-block so the
            # all-gather hides and PE bubbles (exp-bound attention) fill up
            qts = {}
            for c in range(NCH):
                qts[c] = qt_p.tile([128, QH * CH], bf16, tag="qt", name=f"qt{c}")
                proj(c, qts[c])
                if debug_dumps and c == 0:
                    nc.sync.dma_start(dbg["qt0"].ap(), qts[c][:])
                attn(c, qts.pop(c), wo_steps(c - 1) if c >= 1 else None)
            for step in wo_steps(NCH - 1):
                step()
            if debug_dumps:
                nc.sync.dma_start(dbg["kt0"].ap(), kT_sb[0][:])
                nc.sync.dma_start(dbg["v"].ap(), v_sb[:])

    nc.compile()
    return nc


# ---------------------------------------------------------------- host side
def _rope_cache():
    fi = np.arange(0, D, 2, dtype=np.float32)
    inv = 1.0 / 10000.0 ** (fi / D)
    ang = np.outer(np.arange(S, dtype=np.float32), inv)  # (S, 64)
    cos = np.concatenate([np.cos(ang)] * 2, -1)          # (S, 128)
    sin = np.sin(ang)
    sinS = np.concatenate([-sin, sin], -1)               # signed
    return (np.ascontiguousarray(cos.T).astype(BF16),
            np.ascontiguousarray(sinS.T).astype(BF16))


def _plan_from_mask(mask):
    """Returns (plan, dmask_per_batch[b] -> np[nm,128,CH] bf16)."""
    m = np.asarray(mask[:, 0])                    # (B, S, S) bool, [q, k]
    tril = np.tril(np.ones((S, S), bool))
    if all(np.array_equal(m[b], tril) for b in range(B)):
        plan = []
        for c in range(NCH):
            blk = [(t, None) for t in range(4 * c)]
            blk += [(4 * c + tt, tt) for tt in range(4)]
            plan.append(blk)
        dm = np.zeros((4, KB, CH), np.float32)
        for tt in range(4):
            for p in range(KB):
                dm[tt, p, tt * KB + p:] = 1.0
        dms = [dm.astype(BF16)] * B
        return plan, dms
    if m.all():
        plan = [[(t, None) for t in range(S // KB)] for _ in range(NCH)]
        z = np.zeros((1, KB, CH), BF16)
        return plan, [z] * B
    # generic: classify blocks against the union across batches
    mT = [np.ascontiguousarray(m[b].T) for b in range(B)]  # [k, q]
    plan, tiles = [], [[] for _ in range(B)]
    nm = 0
    for c in range(NCH):
        blk = []
        for t in range(S // KB):
            subs = [mT[b][t * KB:(t + 1) * KB, c * CH:(c + 1) * CH]
                    for b in range(B)]
            if all(not s.any() for s in subs):
                continue
            if all(s.all() for s in subs):
                blk.append((t, None))
            else:
                blk.append((t, nm))
                for b in range(B):
                    tiles[b].append(subs[b].astype(BF16))
                nm += 1
        plan.append(blk)
    dms = [np.stack(tiles[b]) if nm else np.zeros((1, KB, CH), BF16)
           for b in range(B)]
    return plan, dms


def _pack_ip(w, nj):
    """[HID, nj*d] -> [nj, 128, NIT*d] tile-packed (i along free)."""
    hid, cols = w.shape
    d = cols // nj
    r = w.reshape(NIT, 128, nj, d).transpose(2, 1, 0, 3)
    return np.ascontiguousarray(r.reshape(nj, 128, NIT * d))


def _prep_inputs(x, mask, Wq, Wk, Wv, Wo):
    cosT, sinS = _rope_cache()
    plan, dms = _plan_from_mask(mask)
    dms_packed = []
    for b in range(B):
        dm = dms[b]  # [nm, 128, 512]
        dms_packed.append(np.ascontiguousarray(
            dm.transpose(1, 0, 2).reshape(KB, -1)))
    xp = {}
    for b in range(B):
        xb = np.ascontiguousarray(x[b].T).astype(BF16)      # [HID, S]
        r = xb.reshape(NIT, 128, NCH, CH).transpose(2, 1, 0, 3)
        xp[b] = np.ascontiguousarray(r.reshape(NCH, 128, NIT * CH))
    in_maps = []
    for c in range(NCORES):
        b, g = c // GRP, c % GRP
        wq_g = Wq[:, g * QD:(g + 1) * QD].astype(BF16)
        wk_g = Wk[:, g * KVD:(g + 1) * KVD].astype(BF16)
        wv_g = Wv[:, g * KVD:(g + 1) * KVD].astype(BF16)
        wo_g = Wo[:, g * QD:(g + 1) * QD].astype(BF16)
        in_maps.append({
            "xT": xp[b],
            "wq": _pack_ip(wq_g, QH),
            "wk": _pack_ip(wk_g, 1)[0],
            "wv": _pack_ip(wv_g, 1)[0],
            "wo": _pack_ip(wo_g, QH),
            "cosT": cosT,
            "sinS": sinS,
            "dmask": dms_packed[b],
        })
    return plan, in_maps


def _get_nc(plan, nmask, debug_dumps=False):
    key = (tuple(tuple(blk) for blk in plan), nmask, debug_dumps)
    if key not in _CACHE:
        _CACHE[key] = build_nc(plan, nmask, debug_dumps)
    return _CACHE[key]


def run(x, mask, Wq, Wk, Wv, Wo, trace=False, debug_dumps=False):
    plan, in_maps = _prep_inputs(x, mask, Wq, Wk, Wv, Wo)
    nmask = in_maps[0]["dmask"].shape[1] // CH
    nc = _get_nc(plan, nmask, debug_dumps)
    res = bass_utils.run_bass_kernel_spmd(
        nc, in_maps, core_ids=list(range(NCORES)), trace=trace)
    out = np.empty((B, S, HID), np.float32)
    for c in range(NCORES):
        b, g = c // GRP, c % GRP
        out[b, :, g * QD:(g + 1) * QD] = res.results[c]["outT"].T
    return out, res


def kernel(x, mask, Wq, Wk, Wv, Wo):
    out, _ = run(np.asarray(x), np.asarray(mask), np.asarray(Wq),
                 np.asarray(Wk), np.asarray(Wv), np.asarray(Wo))
    return out


# needed only when profiling (trace=True) inside this container
def install_ntff_hook():
    try:
        from antenv.axon_hooks import get_axon_ntff_profile_hook  # noqa: F401
        return
    except ImportError:
        pass
    import types
    import antenv
    try:
        from trn_agent_boot.trn_boot import _ntff_profile_via_ctypes
        hook = _ntff_profile_via_ctypes('/opt/axon/libaxon_pjrt.so')
    except Exception:
        hook = None
    mod = types.ModuleType("antenv.axon_hooks")
    state = {"h": hook}
    mod.get_axon_ntff_profile_hook = lambda: state["h"]
    mod.set_axon_ntff_profile_hook = lambda h: state.__setitem__("h", h)
    sys.modules["antenv.axon_hooks"] = mod
    antenv.axon_hooks = mod


install_ntff_hook()
bass_utils.upload_artifacts = lambda tmpdir: "local://" + str(tmpdir)


# revision 25
# speedup vs baseline: 1.3138x; 1.0732x over previous
"""Trainium2 Bass kernel for GQA attention (dense_transformer).

Sharding: 8 cores = 2-way data parallel (batch) x 4-way tensor parallel (heads).
Core c handles batch b=c//4, head group g=c%4 (8 q heads, 2 kv heads).
Per core: QKV projections (bf16, f32 accum), RoPE, causal attention with
transposed-scores layout (scoresT[k,q] -> probsT used directly as the moving
operand of the PV matmul; no on-chip transposes), per-chunk AllGather of
attention outputs within each 4-core group, then a column-sharded Wo matmul.
Host assembles disjoint output shards (pure unshard, no host math).

All inputs are host-packed into the exact SBUF tile layouts so every DMA is a
plain 2D transfer (contiguous per partition).
"""
import sys

sys.path.insert(0, "/opt/trn_rl_repo")

import numpy as np
import ml_dtypes

import concourse.bacc as bacc
import concourse.mybir as mybir
import concourse.tile as tile
from concourse import bass_utils

BF16 = ml_dtypes.bfloat16

B, S, HID = 2, 2048, 4096
NH, NKV, D = 32, 8, 128
NCORES, GRP = 8, 4          # 2 groups of 4 cores
QH, KVH = NH // GRP, NKV // GRP   # 8 q heads, 2 kv heads per core
QD, KVD = QH * D, KVH * D         # 1024, 256
CH, NCH = 512, S // 512           # q-chunk size / count
KB = 128                          # k block
NIT = HID // 128                  # 32 contraction tiles
SM_SCALE = float(D) ** -0.5
REPLICA_GROUPS = [[0, 1, 2, 3], [4, 5, 6, 7]]

_CACHE: dict = {}


# ---------------------------------------------------------------- builder
def build_nc(plan, nmask, debug_dumps=False):
    """plan[c] = list of (t, mask_idx_or_None) k-blocks for q-chunk c."""
    fp32, bf16, f32r = mybir.dt.float32, mybir.dt.bfloat16, mybir.dt.float32r
    nc = bacc.Bacc("TRN2", target_bir_lowering=False, debug=False,
                   num_devices=NCORES)

    # host-packed inputs (exact SBUF layouts; all DMAs contiguous/partition)
    xT = nc.dram_tensor("xT", [NCH, 128, NIT * CH], bf16, kind="ExternalInput")
    wq = nc.dram_tensor("wq", [QH, 128, NIT * 128], bf16, kind="ExternalInput")
    wk = nc.dram_tensor("wk", [128, NIT * KVD], bf16, kind="ExternalInput")
    wv = nc.dram_tensor("wv", [128, NIT * KVD], bf16, kind="ExternalInput")
    wo = nc.dram_tensor("wo", [QH, 128, NIT * 128], bf16, kind="ExternalInput")
    cosT = nc.dram_tensor("cosT", [D, S], bf16, kind="ExternalInput")
    sinS = nc.dram_tensor("sinS", [D, S], bf16, kind="ExternalInput")
    nm = max(nmask, 1)
    dmask = nc.dram_tensor("dmask", [KB, nm * CH], bf16, kind="ExternalInput")
    outT = nc.dram_tensor("outT", [QD, S], fp32, kind="ExternalOutput")

    # partition-major bounce buffers, split in two head-halves per chunk so
    # each half-gather can issue as soon as its 4 heads are done; gather
    # concatenates the 4 group members along dim0 -> [GRP*128, (QH/2)*CH]
    HH = QH // 2
    bnc_in = [[nc.dram_tensor(f"bnc_in{c}_{hf}", [128, HH * CH], bf16)
               for hf in range(2)] for c in range(NCH)]
    bnc_out = [[nc.dram_tensor(f"bnc_out{c}_{hf}", [GRP * 128, HH * CH], bf16)
                for hf in range(2)] for c in range(NCH)]

    dbg = {}
    if debug_dumps:
        dbg["qt0"] = nc.dram_tensor("dbg_qt0", [128, QH * CH], bf16,
                                    kind="ExternalOutput")
        dbg["kt0"] = nc.dram_tensor("dbg_kt0", [D, S], bf16,
                                    kind="ExternalOutput")
        dbg["v"] = nc.dram_tensor("dbg_v", [128, (S // 128) * KVD], bf16,
                                  kind="ExternalOutput")

    with tile.TileContext(nc) as tc:
        from contextlib import ExitStack
        with ExitStack() as ctx:
            P = lambda **kw: ctx.enter_context(tc.tile_pool(**kw))
            const_p = P(name="const", bufs=1)
            wkv_p = P(name="wkv", bufs=1)
            res_p = P(name="res", bufs=1)         # kT, v, cos, sin, masks
            xt_p = P(name="xt", bufs=1)
            wq_p = P(name="wqp", bufs=2)
            qt_p = P(name="qt", bufs=2)
            rope_p = P(name="rope", bufs=2)
            probs_p = P(name="probs", bufs=3)
            attn_p = P(name="attn", bufs=3)
            gath_p = P(name="gath", bufs=1)
            wo_p = P(name="wop", bufs=2)
            out_p = P(name="outp", bufs=2)
            recip_p = P(name="recip", bufs=2)
            pA = P(name="pA", bufs=2, space="PSUM")
            psc = P(name="psc", bufs=2, space="PSUM")
            po = P(name="po", bufs=2, space="PSUM")
            psums = P(name="psums", bufs=1, space="PSUM")
            pbc = P(name="pbc", bufs=1, space="PSUM")

            # constants
            ones_f = const_p.tile([1, 128], fp32, tag="ones_f")
            nc.gpsimd.memset(ones_f[:], 1.0)
            ones_r = const_p.tile([1, 128], f32r, tag="ones_r")
            nc.vector.tensor_copy(ones_r[:], ones_f[:])
            ones_bf = const_p.tile([128, 1], bf16, tag="ones_bf")
            nc.gpsimd.memset(ones_bf[:], 1.0)
            ones_cf = const_p.tile([128, 1], fp32, tag="ones_cf")
            nc.gpsimd.memset(ones_cf[:], 1.0)
            ones_cr = const_p.tile([128, 1], f32r, tag="ones_cr")
            nc.vector.tensor_copy(ones_cr[:], ones_cf[:])

            # resident loads (all straight 2D)
            wk_sb = wkv_p.tile([128, NIT * KVD], bf16, tag="wk")
            nc.sync.dma_start(wk_sb[:], wk.ap())
            wv_sb = wkv_p.tile([128, NIT * KVD], bf16, tag="wv")
            nc.sync.dma_start(wv_sb[:], wv.ap())
            cos_sb = res_p.tile([D, S], bf16, tag="cos")
            nc.sync.dma_start(cos_sb[:], cosT.ap())
            sin_sb = res_p.tile([D, S], bf16, tag="sin")
            nc.sync.dma_start(sin_sb[:], sinS.ap())
            dm_sb = res_p.tile([KB, nm * CH], bf16, tag="dm")
            nc.sync.dma_start(dm_sb[:], dmask.ap())
            kT_sb = [res_p.tile([D, S], bf16, tag=f"kT{kj}", name=f"kT{kj}")
                     for kj in range(KVH)]
            v_sb = res_p.tile([128, (S // 128) * KVD], bf16, tag="v")

            Exp = mybir.ActivationFunctionType.Exp

            def rope(dst, ps, c):
                """dst (bf16 [128,CH]) = rope(ps) with cos/sin chunk c."""
                cs = cos_sb[:, c * CH:(c + 1) * CH]
                sn = sin_sb[:, c * CH:(c + 1) * CH]
                m1 = rope_p.tile([128, CH], fp32, tag="m1")
                m2 = rope_p.tile([128, CH], fp32, tag="m2")
                nc.vector.tensor_mul(m1[:], ps[:], cs)
                nc.vector.tensor_mul(m2[0:64, :], ps[64:128, :], sn[0:64, :])
                nc.vector.tensor_mul(m2[64:128, :], ps[0:64, :], sn[64:128, :])
                nc.vector.tensor_add(dst, m1[:], m2[:])

            def proj(c, qt):
                xt = xt_p.tile([128, NIT * CH], bf16, tag="xt")
                nc.sync.dma_start(xt[:], xT.ap()[c])
                for j in range(QH):
                    wq_t = wq_p.tile([128, NIT * 128], bf16, tag="wq")
                    nc.sync.dma_start(wq_t[:], wq.ap()[j])
                    ps = pA.tile([128, CH], fp32, tag="pA")
                    for i in range(NIT):
                        nc.tensor.matmul(
                            ps[:], wq_t[:, i * 128:(i + 1) * 128],
                            xt[:, i * CH:(i + 1) * CH],
                            start=(i == 0), stop=(i == NIT - 1))
                    rope(qt[:, j * CH:(j + 1) * CH], ps, c)
                for kj in range(KVH):
                    ps = pA.tile([128, CH], fp32, tag="pA")
                    for i in range(NIT):
                        nc.tensor.matmul(
                            ps[:], wk_sb[:, i * KVD + kj * 128:
                                         i * KVD + (kj + 1) * 128],
                            xt[:, i * CH:(i + 1) * CH],
                            start=(i == 0), stop=(i == NIT - 1))
                    rope(kT_sb[kj][:, c * CH:(c + 1) * CH], ps, c)
                for t in range(CH // 128):
                    ps = pA.tile([128, KVD], fp32, tag="pA")
                    for i in range(NIT):
                        nc.tensor.matmul(
                            ps[:], xt[:, i * CH + t * 128:i * CH + (t + 1) * 128],
                            wv_sb[:, i * KVD:(i + 1) * KVD],
                            start=(i == 0), stop=(i == NIT - 1))
                    sb = (c * (CH // 128) + t) * KVD
                    nc.scalar.copy(v_sb[:, sb:sb + KVD], ps[:])

            # deferred normalization tail: after the last ones-matmul of head
            # h, only the (cheap) reciprocal is emitted inline; the broadcast
            # matmul + final multiply + DMA run while head h+1's scores are
            # already streaming on PE.
            def make_tail(c, h, po_t, su_t):
                rc = recip_p.tile([1, CH], fp32, tag="rc", name=f"rc{c}_{h}")
                nc.vector.reciprocal_approx_fast(rc[:], su_t[:])
                rr = recip_p.tile([1, CH], f32r, tag="rr", name=f"rr{c}_{h}")
                nc.vector.tensor_copy(rr[:], rc[:])

                def flush():
                    pb = pbc.tile([128, CH], fp32, tag="pb")
                    nc.tensor.matmul(pb[:], ones_r[:], rr[:], start=True,
                                     stop=True, skip_group_check=True)
                    pbs = recip_p.tile([128, CH], fp32, tag="pbs")
                    nc.scalar.copy(pbs[:], pb[:])
                    at = attn_p.tile([128, CH], bf16, tag="at")
                    nc.vector.tensor_mul(at[:], po_t[:], pbs[:])
                    nc.scalar.dma_start(
                        bnc_in[c][h // HH].ap()[:, (h % HH) * CH:
                                                (h % HH + 1) * CH], at[:])
                return flush

            def wo_steps(c):
                """Generator of emission closures for the Wo phase of chunk c
                (interleaved into the next chunk's attention to fill PE
                bubbles left by the exp chain). Contraction runs half-0 hid
                tiles first so only the first half-gather gates the start."""
                gts = []
                for hf in range(2):
                    gt = gath_p.tile([128, GRP * HH * CH], bf16, tag=f"gt{hf}",
                                     name=f"gt{c}_{hf}")
                    nc.scalar.dma_start(
                        gt[:].rearrange("p (g f) -> p g f", g=GRP),
                        bnc_out[c][hf].ap().rearrange("(g p) f -> p g f",
                                                      p=128))
                    gts.append(gt)
                # hid tile H = cg*QH + h ; half hf = h // HH
                order = ([(cg * QH + h) for h in range(HH)
                          for cg in range(GRP)],
                         [(cg * QH + h) for h in range(HH, QH)
                          for cg in range(GRP)])
                for oj in range(QH):
                    wo_t = wo_p.tile([128, NIT * 128], bf16, tag="wo",
                                     name=f"wo{c}_{oj}")
                    nc.sync.dma_start(wo_t[:], wo.ap()[oj])
                    ps = pA.tile([128, CH], fp32, tag="pA",
                                 name=f"wops{c}_{oj}")
                    nmm = 0
                    for hf in range(2):
                        for H in order[hf]:
                            cg, h = H // QH, H % QH
                            fo = (cg * HH + (h % HH)) * CH
                            def mm(H=H, hf=hf, fo=fo, ps=ps, wo_t=wo_t,
                                   nmm=nmm):
                                nc.tensor.matmul(
                                    ps[:], wo_t[:, H * 128:(H + 1) * 128],
                                    gts[hf][:, fo:fo + CH],
                                    start=(nmm == 0), stop=(nmm == NIT - 1),
                                    skip_group_check=True)
                            yield mm
                            nmm += 1

                    def fin(ps=ps, oj=oj):
                        ot = out_p.tile([128, CH], fp32, tag="ot")
                        nc.vector.tensor_copy(ot[:], ps[:])
                        nc.scalar.dma_start(
                            outT.ap()[oj * 128:(oj + 1) * 128,
                                      c * CH:(c + 1) * CH], ot[:])
                    yield fin

            def attn(c, qt, wo_iter):
                blocks = plan[c]
                nb = len(blocks)
                # one wo matmul per attention block roughly balances PE
                # against the exp-bound ACT chain; the rest drains after
                wo_per_block = 1
                pending = None
                pending_half = None

                def gather(hf):
                    nc.gpsimd.collective_compute(
                        "AllGather", mybir.AluOpType.bypass,
                        replica_groups=REPLICA_GROUPS,
                        ins=[bnc_in[c][hf].ap().opt()],
                        outs=[bnc_out[c][hf].ap().opt()])
                for h in range(QH):
                    kvh = h // (QH // KVH)
                    qs = qt[:, h * CH:(h + 1) * CH]
                    po_t = po.tile([128, CH], fp32, tag="po")
                    su_t = psums.tile([1, CH], fp32, tag="su")
                    acc = attn_p.tile([128, CH], f32r, tag="acc", bufs=2)
                    sc_tiles = {}

                    def emit_sc(bi):
                        t, _ = blocks[bi]
                        p = psc.tile([128, CH], fp32, tag="psc")
                        nc.tensor.matmul(
                            p[:], kT_sb[kvh][:, t * KB:(t + 1) * KB], qs,
                            start=True, stop=True, skip_group_check=True)
                        sc_tiles[bi] = p

                    emit_sc(0)
                    for bi in range(nb):
                        if bi + 1 < nb:
                            emit_sc(bi + 1)
                        if bi == 1 and pending is not None:
                            pending()
                            pending = None
                            if pending_half is not None:
                                gather(pending_half)
                                pending_half = None
                        t, mi = blocks[bi]
                        p = sc_tiles.pop(bi)
                        pr = probs_p.tile([KB, CH], bf16, tag="pr")
                        nc.scalar.activation(pr[:], p[:], Exp, scale=SM_SCALE)
                        if mi is not None:
                            nc.vector.tensor_mul(
                                pr[:], pr[:], dm_sb[:, mi * CH:(mi + 1) * CH])
                        first, last = (bi == 0), (bi == nb - 1)
                        vsl = v_sb[:, t * KVD + kvh * 128:t * KVD + (kvh + 1) * 128]
                        nc.tensor.matmul(po_t[:], vsl, pr[:], start=first,
                                         stop=last, skip_group_check=True)
                        if first:
                            nc.vector.tensor_copy(acc[:], pr[:])
                        else:
                            nc.vector.tensor_add(acc[:], acc[:], pr[:])
                        if wo_iter is not None:
                            for _ in range(wo_per_block):
                                step = next(wo_iter, None)
                                if step is None:
                                    wo_iter = None
                                    break
                                step()
                    # single ones-matmul on the accumulated probs
                    nc.tensor.matmul(su_t[:], ones_cr[:], acc[:], start=True,
                                     stop=True, skip_group_check=True)
                    if pending is not None:
                        pending()
                        if pending_half is not None:
                            gather(pending_half)
                            pending_half = None
                    pending = make_tail(c, h, po_t, su_t)
                    if h == HH - 1:
                        pending_half = 0
                pending()
                gather(1)
                if wo_iter is not None:
                    for step in wo_iter:
                        step()

            # pipeline: wo(c) interleaved into attn(c+1) block-by-block so the
            # all-gather hides and PE bubbles (exp-bound attention) fill up
            qts = {}
            for c in range(NCH):
                qts[c] = qt_p.tile([128, QH * CH], bf16, tag="qt", name=f"qt{c}")
                proj(c, qts[c])
                if debug_dumps and c == 0:
                    nc.sync.dma_start(dbg["qt0"].ap(), qts[c][:])
                attn(c, qts.pop(c), wo_steps(c - 1) if c >= 1 else None)
            for step in wo_steps(NCH - 1):
                step()
            if debug_dumps:
                nc.sync.dma_start(dbg["kt0"].ap(), kT_sb[0][:])
                nc.sync.dma_start(dbg["v"].ap(), v_sb[:])

    nc.compile()
    return nc


# ---------------------------------------------------------------- host side
def _rope_cache():
    fi = np.arange(0, D, 2, dtype=np.float32)
    inv = 1.0 / 10000.0 ** (fi / D)
    ang = np.outer(np.arange(S, dtype=np.float32), inv)  # (S, 64)
    cos = np.concatenate([np.cos(ang)] * 2, -1)          # (S, 128)
    sin = np.sin(ang)
    sinS = np.concatenate([-sin, sin], -1)               # signed
    return (np.ascontiguousarray(cos.T).astype(BF16),
            np.ascontiguousarray(sinS.T).astype(BF16))


def _plan_from_mask(mask):
    """Returns (plan, dmask_per_batch[b] -> np[nm,128,CH] bf16)."""
    m = np.asarray(mask[:, 0])                    # (B, S, S) bool, [q, k]
    tril = np.tril(np.ones((S, S), bool))
    if all(np.array_equal(m[b], tril) for b in range(B)):
        plan = []
        for c in range(NCH):
            blk = [(t, None) for t in range(4 * c)]
            blk += [(4 * c + tt, tt) for tt in range(4)]
            plan.append(blk)
        dm = np.zeros((4, KB, CH), np.float32)
        for tt in range(4):
            for p in range(KB):
                dm[tt, p, tt * KB + p:] = 1.0
        dms = [dm.astype(BF16)] * B
        return plan, dms
    if m.all():
        plan = [[(t, None) for t in range(S // KB)] for _ in range(NCH)]
        z = np.zeros((1, KB, CH), BF16)
        return plan, [z] * B
    # generic: classify blocks against the union across batches
    mT = [np.ascontiguousarray(m[b].T) for b in range(B)]  # [k, q]
    plan, tiles = [], [[] for _ in range(B)]
    nm = 0
    for c in range(NCH):
        blk = []
        for t in range(S // KB):
            subs = [mT[b][t * KB:(t + 1) * KB, c * CH:(c + 1) * CH]
                    for b in range(B)]
            if all(not s.any() for s in subs):
                continue
            if all(s.all() for s in subs):
                blk.append((t, None))
            else:
                blk.append((t, nm))
                for b in range(B):
                    tiles[b].append(subs[b].astype(BF16))
                nm += 1
        plan.append(blk)
    dms = [np.stack(tiles[b]) if nm else np.zeros((1, KB, CH), BF16)
           for b in range(B)]
    return plan, dms


def _pack_ip(w, nj):
    """[HID, nj*d] -> [nj, 128, NIT*d] tile-packed (i along free)."""
    hid, cols = w.shape
    d = cols // nj
    r = w.reshape(NIT, 128, nj, d).transpose(2, 1, 0, 3)
    return np.ascontiguousarray(r.reshape(nj, 128, NIT * d))


def _prep_inputs(x, mask, Wq, Wk, Wv, Wo):
    cosT, sinS = _rope_cache()
    plan, dms = _plan_from_mask(mask)
    dms_packed = []
    for b in range(B):
        dm = dms[b]  # [nm, 128, 512]
        dms_packed.append(np.ascontiguousarray(
            dm.transpose(1, 0, 2).reshape(KB, -1)))
    xp = {}
    for b in range(B):
        xb = np.ascontiguousarray(x[b].T).astype(BF16)      # [HID, S]
        r = xb.reshape(NIT, 128, NCH, CH).transpose(2, 1, 0, 3)
        xp[b] = np.ascontiguousarray(r.reshape(NCH, 128, NIT * CH))
    in_maps = []
    for c in range(NCORES):
        b, g = c // GRP, c % GRP
        wq_g = Wq[:, g * QD:(g + 1) * QD].astype(BF16)
        wk_g = Wk[:, g * KVD:(g + 1) * KVD].astype(BF16)
        wv_g = Wv[:, g * KVD:(g + 1) * KVD].astype(BF16)
        wo_g = Wo[:, g * QD:(g + 1) * QD].astype(BF16)
        in_maps.append({
            "xT": xp[b],
            "wq": _pack_ip(wq_g, QH),
            "wk": _pack_ip(wk_g, 1)[0],
            "wv": _pack_ip(wv_g, 1)[0],
            "wo": _pack_ip(wo_g, QH),
            "cosT": cosT,
            "sinS": sinS,
            "dmask": dms_packed[b],
        })
    return plan, in_maps


def _get_nc(plan, nmask, debug_dumps=False):
    key = (tuple(tuple(blk) for blk in plan), nmask, debug_dumps)
    if key not in _CACHE:
        _CACHE[key] = build_nc(plan, nmask, debug_dumps)
    return _CACHE[key]


def run(x, mask, Wq, Wk, Wv, Wo, trace=False, debug_dumps=False):
    plan, in_maps = _prep_inputs(x, mask, Wq, Wk, Wv, Wo)
    nmask = in_maps[0]["dmask"].shape[1] // CH
    nc = _get_nc(plan, nmask, debug_dumps)
    res = bass_utils.run_bass_kernel_spmd(
        nc, in_maps, core_ids=list(range(NCORES)), trace=trace)
    out = np.empty((B, S, HID), np.float32)
    for c in range(NCORES):
        b, g = c // GRP, c % GRP
        out[b, :, g * QD:(g + 1) * QD] = res.results[c]["outT"].T
    return out, res


def kernel(x, mask, Wq, Wk, Wv, Wo):
    out, _ = run(np.asarray(x), np.asarray(mask), np.asarray(Wq),
                 np.asarray(Wk), np.asarray(Wv), np.asarray(Wo))
    return out


# needed only when profiling (trace=True) inside this container
def install_ntff_hook():
    try:
        from antenv.axon_hooks import get_axon_ntff_profile_hook  # noqa: F401
        return
    except ImportError:
        pass
    import types
    import antenv
    try:
        from trn_agent_boot.trn_boot import _ntff_profile_via_ctypes
        hook = _ntff_profile_via_ctypes('/opt/axon/libaxon_pjrt.so')
    except Exception:
        hook = None
    mod = types.ModuleType("antenv.axon_hooks")
    state = {"h": hook}
    mod.get_axon_ntff_profile_hook = lambda: state["h"]
    mod.set_axon_ntff_profile_hook = lambda h: state.__setitem__("h", h)
    sys.modules["antenv.axon_hooks"] = mod
    antenv.axon_hooks = mod


install_ntff_hook()
bass_utils.upload_artifacts = lambda tmpdir: "local://" + str(tmpdir)


# revision 26
# speedup vs baseline: 1.3158x; 1.0015x over previous
"""Trainium2 Bass kernel for GQA attention (dense_transformer).

Sharding: 8 cores = 2-way data parallel (batch) x 4-way tensor parallel (heads).
Core c handles batch b=c//4, head group g=c%4 (8 q heads, 2 kv heads).
Per core: QKV projections (bf16, f32 accum), RoPE, causal attention with
transposed-scores layout (scoresT[k,q] -> probsT used directly as the moving
operand of the PV matmul; no on-chip transposes), per-chunk AllGather of
attention outputs within each 4-core group, then a column-sharded Wo matmul.
Host assembles disjoint output shards (pure unshard, no host math).

All inputs are host-packed into the exact SBUF tile layouts so every DMA is a
plain 2D transfer (contiguous per partition).
"""
import sys

sys.path.insert(0, "/opt/trn_rl_repo")

import numpy as np
import ml_dtypes

import concourse.bacc as bacc
import concourse.mybir as mybir
import concourse.tile as tile
from concourse import bass_utils

BF16 = ml_dtypes.bfloat16

B, S, HID = 2, 2048, 4096
NH, NKV, D = 32, 8, 128
NCORES, GRP = 8, 4          # 2 groups of 4 cores
QH, KVH = NH // GRP, NKV // GRP   # 8 q heads, 2 kv heads per core
QD, KVD = QH * D, KVH * D         # 1024, 256
CH, NCH = 512, S // 512           # q-chunk size / count
KB = 128                          # k block
NIT = HID // 128                  # 32 contraction tiles
SM_SCALE = float(D) ** -0.5
REPLICA_GROUPS = [[0, 1, 2, 3], [4, 5, 6, 7]]

_CACHE: dict = {}


# ---------------------------------------------------------------- builder
def build_nc(plan, nmask, debug_dumps=False):
    """plan[c] = list of (t, mask_idx_or_None) k-blocks for q-chunk c."""
    fp32, bf16, f32r = mybir.dt.float32, mybir.dt.bfloat16, mybir.dt.float32r
    nc = bacc.Bacc("TRN2", target_bir_lowering=False, debug=False,
                   num_devices=NCORES)

    # host-packed inputs (exact SBUF layouts; all DMAs contiguous/partition)
    xT = nc.dram_tensor("xT", [NCH, 128, NIT * CH], bf16, kind="ExternalInput")
    wq = nc.dram_tensor("wq", [QH, 128, NIT * 128], bf16, kind="ExternalInput")
    wk = nc.dram_tensor("wk", [128, NIT * KVD], bf16, kind="ExternalInput")
    wv = nc.dram_tensor("wv", [128, NIT * KVD], bf16, kind="ExternalInput")
    wo = nc.dram_tensor("wo", [QH, 128, NIT * 128], bf16, kind="ExternalInput")
    cosT = nc.dram_tensor("cosT", [D, S], bf16, kind="ExternalInput")
    sinS = nc.dram_tensor("sinS", [D, S], bf16, kind="ExternalInput")
    nm = max(nmask, 1)
    dmask = nc.dram_tensor("dmask", [KB, nm * CH], bf16, kind="ExternalInput")
    outT = nc.dram_tensor("outT", [QD, S], fp32, kind="ExternalOutput")

    # partition-major bounce buffers, split in two head-halves per chunk so
    # each half-gather can issue as soon as its 4 heads are done; gather
    # concatenates the 4 group members along dim0 -> [GRP*128, (QH/2)*CH]
    HH = QH // 2
    bnc_in = [[nc.dram_tensor(f"bnc_in{c}_{hf}", [128, HH * CH], bf16)
               for hf in range(2)] for c in range(NCH)]
    bnc_out = [[nc.dram_tensor(f"bnc_out{c}_{hf}", [GRP * 128, HH * CH], bf16)
                for hf in range(2)] for c in range(NCH)]

    dbg = {}
    if debug_dumps:
        dbg["qt0"] = nc.dram_tensor("dbg_qt0", [128, QH * CH], bf16,
                                    kind="ExternalOutput")
        dbg["kt0"] = nc.dram_tensor("dbg_kt0", [D, S], bf16,
                                    kind="ExternalOutput")
        dbg["v"] = nc.dram_tensor("dbg_v", [128, (S // 128) * KVD], bf16,
                                  kind="ExternalOutput")

    with tile.TileContext(nc) as tc:
        from contextlib import ExitStack
        with ExitStack() as ctx:
            P = lambda **kw: ctx.enter_context(tc.tile_pool(**kw))
            const_p = P(name="const", bufs=1)
            wkv_p = P(name="wkv", bufs=1)
            res_p = P(name="res", bufs=1)         # kT, v, cos, sin, masks
            xt_p = P(name="xt", bufs=1)
            wq_p = P(name="wqp", bufs=2)
            qt_p = P(name="qt", bufs=2)
            rope_p = P(name="rope", bufs=2)
            probs_p = P(name="probs", bufs=3)
            attn_p = P(name="attn", bufs=3)
            gath_p = P(name="gath", bufs=1)
            wo_p = P(name="wop", bufs=2)
            out_p = P(name="outp", bufs=2)
            recip_p = P(name="recip", bufs=2)
            pA = P(name="pA", bufs=2, space="PSUM")
            psc = P(name="psc", bufs=2, space="PSUM")
            po = P(name="po", bufs=2, space="PSUM")
            psums = P(name="psums", bufs=1, space="PSUM")
            pbc = P(name="pbc", bufs=1, space="PSUM")

            # constants
            ones_f = const_p.tile([1, 128], fp32, tag="ones_f")
            nc.gpsimd.memset(ones_f[:], 1.0)
            ones_r = const_p.tile([1, 128], f32r, tag="ones_r")
            nc.vector.tensor_copy(ones_r[:], ones_f[:])
            ones_bf = const_p.tile([128, 1], bf16, tag="ones_bf")
            nc.gpsimd.memset(ones_bf[:], 1.0)
            ones_cf = const_p.tile([128, 1], fp32, tag="ones_cf")
            nc.gpsimd.memset(ones_cf[:], 1.0)
            ones_cr = const_p.tile([128, 1], f32r, tag="ones_cr")
            nc.vector.tensor_copy(ones_cr[:], ones_cf[:])

            # resident loads (all straight 2D)
            wk_sb = wkv_p.tile([128, NIT * KVD], bf16, tag="wk")
            nc.scalar.dma_start(wk_sb[:], wk.ap())
            wv_sb = wkv_p.tile([128, NIT * KVD], bf16, tag="wv")
            nc.scalar.dma_start(wv_sb[:], wv.ap())
            cos_sb = res_p.tile([D, S], bf16, tag="cos")
            nc.scalar.dma_start(cos_sb[:], cosT.ap())
            sin_sb = res_p.tile([D, S], bf16, tag="sin")
            nc.scalar.dma_start(sin_sb[:], sinS.ap())
            dm_sb = res_p.tile([KB, nm * CH], bf16, tag="dm")
            nc.scalar.dma_start(dm_sb[:], dmask.ap())
            kT_sb = [res_p.tile([D, S], bf16, tag=f"kT{kj}", name=f"kT{kj}")
                     for kj in range(KVH)]
            v_sb = res_p.tile([128, (S // 128) * KVD], bf16, tag="v")

            Exp = mybir.ActivationFunctionType.Exp

            def rope(dst, ps, c):
                """dst (bf16 [128,CH]) = rope(ps) with cos/sin chunk c."""
                cs = cos_sb[:, c * CH:(c + 1) * CH]
                sn = sin_sb[:, c * CH:(c + 1) * CH]
                m1 = rope_p.tile([128, CH], fp32, tag="m1")
                m2 = rope_p.tile([128, CH], fp32, tag="m2")
                nc.vector.tensor_mul(m1[:], ps[:], cs)
                nc.vector.tensor_mul(m2[0:64, :], ps[64:128, :], sn[0:64, :])
                nc.vector.tensor_mul(m2[64:128, :], ps[0:64, :], sn[64:128, :])
                nc.vector.tensor_add(dst, m1[:], m2[:])

            def proj(c, qt):
                xt = xt_p.tile([128, NIT * CH], bf16, tag="xt")
                nc.sync.dma_start(xt[:], xT.ap()[c])
                for j in range(QH):
                    wq_t = wq_p.tile([128, NIT * 128], bf16, tag="wq")
                    nc.sync.dma_start(wq_t[:], wq.ap()[j])
                    ps = pA.tile([128, CH], fp32, tag="pA")
                    for i in range(NIT):
                        nc.tensor.matmul(
                            ps[:], wq_t[:, i * 128:(i + 1) * 128],
                            xt[:, i * CH:(i + 1) * CH],
                            start=(i == 0), stop=(i == NIT - 1))
                    rope(qt[:, j * CH:(j + 1) * CH], ps, c)
                for kj in range(KVH):
                    ps = pA.tile([128, CH], fp32, tag="pA")
                    for i in range(NIT):
                        nc.tensor.matmul(
                            ps[:], wk_sb[:, i * KVD + kj * 128:
                                         i * KVD + (kj + 1) * 128],
                            xt[:, i * CH:(i + 1) * CH],
                            start=(i == 0), stop=(i == NIT - 1))
                    rope(kT_sb[kj][:, c * CH:(c + 1) * CH], ps, c)
                for t in range(CH // 128):
                    ps = pA.tile([128, KVD], fp32, tag="pA")
                    for i in range(NIT):
                        nc.tensor.matmul(
                            ps[:], xt[:, i * CH + t * 128:i * CH + (t + 1) * 128],
                            wv_sb[:, i * KVD:(i + 1) * KVD],
                            start=(i == 0), stop=(i == NIT - 1))
                    sb = (c * (CH // 128) + t) * KVD
                    nc.scalar.copy(v_sb[:, sb:sb + KVD], ps[:])

            # deferred normalization tail: after the last ones-matmul of head
            # h, only the (cheap) reciprocal is emitted inline; the broadcast
            # matmul + final multiply + DMA run while head h+1's scores are
            # already streaming on PE.
            def make_tail(c, h, po_t, su_t):
                rc = recip_p.tile([1, CH], fp32, tag="rc", name=f"rc{c}_{h}")
                nc.vector.reciprocal_approx_fast(rc[:], su_t[:])
                rr = recip_p.tile([1, CH], f32r, tag="rr", name=f"rr{c}_{h}")
                nc.vector.tensor_copy(rr[:], rc[:])

                def flush():
                    pb = pbc.tile([128, CH], fp32, tag="pb")
                    nc.tensor.matmul(pb[:], ones_r[:], rr[:], start=True,
                                     stop=True, skip_group_check=True)
                    pbs = recip_p.tile([128, CH], fp32, tag="pbs")
                    nc.scalar.copy(pbs[:], pb[:])
                    at = attn_p.tile([128, CH], bf16, tag="at")
                    nc.vector.tensor_mul(at[:], po_t[:], pbs[:])
                    nc.scalar.dma_start(
                        bnc_in[c][h // HH].ap()[:, (h % HH) * CH:
                                                (h % HH + 1) * CH], at[:])
                return flush

            def wo_steps(c):
                """Generator of emission closures for the Wo phase of chunk c
                (interleaved into the next chunk's attention to fill PE
                bubbles left by the exp chain). Contraction runs half-0 hid
                tiles first so only the first half-gather gates the start."""
                gts = []
                for hf in range(2):
                    gt = gath_p.tile([128, GRP * HH * CH], bf16, tag=f"gt{hf}",
                                     name=f"gt{c}_{hf}")
                    nc.scalar.dma_start(
                        gt[:].rearrange("p (g f) -> p g f", g=GRP),
                        bnc_out[c][hf].ap().rearrange("(g p) f -> p g f",
                                                      p=128))
                    gts.append(gt)
                # hid tile H = cg*QH + h ; half hf = h // HH
                order = ([(cg * QH + h) for h in range(HH)
                          for cg in range(GRP)],
                         [(cg * QH + h) for h in range(HH, QH)
                          for cg in range(GRP)])
                for oj in range(QH):
                    wo_t = wo_p.tile([128, NIT * 128], bf16, tag="wo",
                                     name=f"wo{c}_{oj}")
                    nc.sync.dma_start(wo_t[:], wo.ap()[oj])
                    ps = pA.tile([128, CH], fp32, tag="pA",
                                 name=f"wops{c}_{oj}")
                    nmm = 0
                    for hf in range(2):
                        for H in order[hf]:
                            cg, h = H // QH, H % QH
                            fo = (cg * HH + (h % HH)) * CH
                            def mm(H=H, hf=hf, fo=fo, ps=ps, wo_t=wo_t,
                                   nmm=nmm):
                                nc.tensor.matmul(
                                    ps[:], wo_t[:, H * 128:(H + 1) * 128],
                                    gts[hf][:, fo:fo + CH],
                                    start=(nmm == 0), stop=(nmm == NIT - 1),
                                    skip_group_check=True)
                            yield mm
                            nmm += 1

                    def fin(ps=ps, oj=oj):
                        ot = out_p.tile([128, CH], fp32, tag="ot")
                        nc.vector.tensor_copy(ot[:], ps[:])
                        nc.scalar.dma_start(
                            outT.ap()[oj * 128:(oj + 1) * 128,
                                      c * CH:(c + 1) * CH], ot[:])
                    yield fin

            def attn(c, qt, wo_iter):
                blocks = plan[c]
                nb = len(blocks)
                # one wo matmul per attention block roughly balances PE
                # against the exp-bound ACT chain; the rest drains after
                wo_per_block = 1
                pending = None
                pending_half = None

                def gather(hf):
                    nc.gpsimd.collective_compute(
                        "AllGather", mybir.AluOpType.bypass,
                        replica_groups=REPLICA_GROUPS,
                        ins=[bnc_in[c][hf].ap().opt()],
                        outs=[bnc_out[c][hf].ap().opt()])
                for h in range(QH):
                    kvh = h // (QH // KVH)
                    qs = qt[:, h * CH:(h + 1) * CH]
                    po_t = po.tile([128, CH], fp32, tag="po")
                    su_t = psums.tile([1, CH], fp32, tag="su")
                    acc = attn_p.tile([128, CH], f32r, tag="acc", bufs=2)
                    sc_tiles = {}

                    def emit_sc(bi):
                        t, _ = blocks[bi]
                        p = psc.tile([128, CH], fp32, tag="psc")
                        nc.tensor.matmul(
                            p[:], kT_sb[kvh][:, t * KB:(t + 1) * KB], qs,
                            start=True, stop=True, skip_group_check=True)
                        sc_tiles[bi] = p

                    emit_sc(0)
                    for bi in range(nb):
                        if bi + 1 < nb:
                            emit_sc(bi + 1)
                        if bi == 1 and pending is not None:
                            pending()
                            pending = None
                            if pending_half is not None:
                                gather(pending_half)
                                pending_half = None
                        t, mi = blocks[bi]
                        p = sc_tiles.pop(bi)
                        pr = probs_p.tile([KB, CH], bf16, tag="pr")
                        nc.scalar.activation(pr[:], p[:], Exp, scale=SM_SCALE)
                        if mi is not None:
                            nc.vector.tensor_mul(
                                pr[:], pr[:], dm_sb[:, mi * CH:(mi + 1) * CH])
                        first, last = (bi == 0), (bi == nb - 1)
                        vsl = v_sb[:, t * KVD + kvh * 128:t * KVD + (kvh + 1) * 128]
                        nc.tensor.matmul(po_t[:], vsl, pr[:], start=first,
                                         stop=last, skip_group_check=True)
                        if first:
                            nc.vector.tensor_copy(acc[:], pr[:])
                        else:
                            nc.vector.tensor_add(acc[:], acc[:], pr[:])
                        if wo_iter is not None and h > 0:
                            for _ in range(wo_per_block):
                                step = next(wo_iter, None)
                                if step is None:
                                    wo_iter = None
                                    break
                                step()
                    # single ones-matmul on the accumulated probs
                    nc.tensor.matmul(su_t[:], ones_cr[:], acc[:], start=True,
                                     stop=True, skip_group_check=True)
                    if pending is not None:
                        pending()
                        if pending_half is not None:
                            gather(pending_half)
                            pending_half = None
                    pending = make_tail(c, h, po_t, su_t)
                    if h == HH - 1:
                        pending_half = 0
                pending()
                gather(1)
                if wo_iter is not None:
                    for step in wo_iter:
                        step()

            # pipeline: wo(c) interleaved into attn(c+1) block-by-block so the
            # all-gather hides and PE bubbles (exp-bound attention) fill up
            qts = {}
            for c in range(NCH):
                qts[c] = qt_p.tile([128, QH * CH], bf16, tag="qt", name=f"qt{c}")
                proj(c, qts[c])
                if debug_dumps and c == 0:
                    nc.sync.dma_start(dbg["qt0"].ap(), qts[c][:])
                attn(c, qts.pop(c), wo_steps(c - 1) if c >= 1 else None)
            for step in wo_steps(NCH - 1):
                step()
            if debug_dumps:
                nc.sync.dma_start(dbg["kt0"].ap(), kT_sb[0][:])
                nc.sync.dma_start(dbg["v"].ap(), v_sb[:])

    nc.compile()
    return nc


# ---------------------------------------------------------------- host side
def _rope_cache():
    fi = np.arange(0, D, 2, dtype=np.float32)
    inv = 1.0 / 10000.0 ** (fi / D)
    ang = np.outer(np.arange(S, dtype=np.float32), inv)  # (S, 64)
    cos = np.concatenate([np.cos(ang)] * 2, -1)          # (S, 128)
    sin = np.sin(ang)
    sinS = np.concatenate([-sin, sin], -1)               # signed
    return (np.ascontiguousarray(cos.T).astype(BF16),
            np.ascontiguousarray(sinS.T).astype(BF16))


def _plan_from_mask(mask):
    """Returns (plan, dmask_per_batch[b] -> np[nm,128,CH] bf16)."""
    m = np.asarray(mask[:, 0])                    # (B, S, S) bool, [q, k]
    tril = np.tril(np.ones((S, S), bool))
    if all(np.array_equal(m[b], tril) for b in range(B)):
        plan = []
        for c in range(NCH):
            blk = [(t, None) for t in range(4 * c)]
            blk += [(4 * c + tt, tt) for tt in range(4)]
            plan.append(blk)
        dm = np.zeros((4, KB, CH), np.float32)
        for tt in range(4):
            for p in range(KB):
                dm[tt, p, tt * KB + p:] = 1.0
        dms = [dm.astype(BF16)] * B
        return plan, dms
    if m.all():
        plan = [[(t, None) for t in range(S // KB)] for _ in range(NCH)]
        z = np.zeros((1, KB, CH), BF16)
        return plan, [z] * B
    # generic: classify blocks against the union across batches
    mT = [np.ascontiguousarray(m[b].T) for b in range(B)]  # [k, q]
    plan, tiles = [], [[] for _ in range(B)]
    nm = 0
    for c in range(NCH):
        blk = []
        for t in range(S // KB):
            subs = [mT[b][t * KB:(t + 1) * KB, c * CH:(c + 1) * CH]
                    for b in range(B)]
            if all(not s.any() for s in subs):
                continue
            if all(s.all() for s in subs):
                blk.append((t, None))
            else:
                blk.append((t, nm))
                for b in range(B):
                    tiles[b].append(subs[b].astype(BF16))
                nm += 1
        plan.append(blk)
    dms = [np.stack(tiles[b]) if nm else np.zeros((1, KB, CH), BF16)
           for b in range(B)]
    return plan, dms


def _pack_ip(w, nj):
    """[HID, nj*d] -> [nj, 128, NIT*d] tile-packed (i along free)."""
    hid, cols = w.shape
    d = cols // nj
    r = w.reshape(NIT, 128, nj, d).transpose(2, 1, 0, 3)
    return np.ascontiguousarray(r.reshape(nj, 128, NIT * d))


def _prep_inputs(x, mask, Wq, Wk, Wv, Wo):
    cosT, sinS = _rope_cache()
    plan, dms = _plan_from_mask(mask)
    dms_packed = []
    for b in range(B):
        dm = dms[b]  # [nm, 128, 512]
        dms_packed.append(np.ascontiguousarray(
            dm.transpose(1, 0, 2).reshape(KB, -1)))
    xp = {}
    for b in range(B):
        xb = np.ascontiguousarray(x[b].T).astype(BF16)      # [HID, S]
        r = xb.reshape(NIT, 128, NCH, CH).transpose(2, 1, 0, 3)
        xp[b] = np.ascontiguousarray(r.reshape(NCH, 128, NIT * CH))
    in_maps = []
    for c in range(NCORES):
        b, g = c // GRP, c % GRP
        wq_g = Wq[:, g * QD:(g + 1) * QD].astype(BF16)
        wk_g = Wk[:, g * KVD:(g + 1) * KVD].astype(BF16)
        wv_g = Wv[:, g * KVD:(g + 1) * KVD].astype(BF16)
        wo_g = Wo[:, g * QD:(g + 1) * QD].astype(BF16)
        in_maps.append({
            "xT": xp[b],
            "wq": _pack_ip(wq_g, QH),
            "wk": _pack_ip(wk_g, 1)[0],
            "wv": _pack_ip(wv_g, 1)[0],
            "wo": _pack_ip(wo_g, QH),
            "cosT": cosT,
            "sinS": sinS,
            "dmask": dms_packed[b],
        })
    return plan, in_maps


def _get_nc(plan, nmask, debug_dumps=False):
    key = (tuple(tuple(blk) for blk in plan), nmask, debug_dumps)
    if key not in _CACHE:
        _CACHE[key] = build_nc(plan, nmask, debug_dumps)
    return _CACHE[key]


def run(x, mask, Wq, Wk, Wv, Wo, trace=False, debug_dumps=False):
    plan, in_maps = _prep_inputs(x, mask, Wq, Wk, Wv, Wo)
    nmask = in_maps[0]["dmask"].shape[1] // CH
    nc = _get_nc(plan, nmask, debug_dumps)
    res = bass_utils.run_bass_kernel_spmd(
        nc, in_maps, core_ids=list(range(NCORES)), trace=trace)
    out = np.empty((B, S, HID), np.float32)
    for c in range(NCORES):
        b, g = c // GRP, c % GRP
        out[b, :, g * QD:(g + 1) * QD] = res.results[c]["outT"].T
    return out, res


def kernel(x, mask, Wq, Wk, Wv, Wo):
    out, _ = run(np.asarray(x), np.asarray(mask), np.asarray(Wq),
                 np.asarray(Wk), np.asarray(Wv), np.asarray(Wo))
    return out


# needed only when profiling (trace=True) inside this container
def install_ntff_hook():
    try:
        from antenv.axon_hooks import get_axon_ntff_profile_hook  # noqa: F401
        return
    except ImportError:
        pass
    import types
    import antenv
    try:
        from trn_agent_boot.trn_boot import _ntff_profile_via_ctypes
        hook = _ntff_profile_via_ctypes('/opt/axon/libaxon_pjrt.so')
    except Exception:
        hook = None
    mod = types.ModuleType("antenv.axon_hooks")
    state = {"h": hook}
    mod.get_axon_ntff_profile_hook = lambda: state["h"]
    mod.set_axon_ntff_profile_hook = lambda h: state.__setitem__("h", h)
    sys.modules["antenv.axon_hooks"] = mod
    antenv.axon_hooks = mod


install_ntff_hook()
bass_utils.upload_artifacts = lambda tmpdir: "local://" + str(tmpdir)
